# revision 10
# baseline (speedup 1.0000x reference)
"""GCN2 Trainium2 kernel: 3-layer GCN + FC head with BatchNorm, 8-core data-parallel.

Self-contained: hardcodes shapes from the problem spec.
  x [256, 128, 65] f32, adj_mat [256, 256] f32, W1 [63, 512], b1 [512],
  W2 [512, 512], b2 [512], W3 [512, 1024], b3 [1024], fcW1 [1024, 512],
  fcb1 [512], gamma [512], beta [512], fcW2 [512, 1], fcb2 [1] -> out [256, 1]

Sharding: batch 256 -> 32 samples per core on 8 cores; weights/adj replicated.
BatchNorm batch stats all-reduced across cores (one small [128,8] AllReduce).

Algorithm (sparse tail):
  The output gather X3[b, g_b] means layers >= 2 are only needed at the
  neighbors of g_b (max degree+1 = R slots per sample). Per sample:
    X1 = relu(An @ scatter(F) @ W1 + b1)          dense [256, 512]
    Z2 = An[nbr(g), :] @ X1                       [R, 512]
    X2 = relu(Z2 @ W2 + b2)                       [R, 512]
    r  = An[g, nbr(g)] @ X2                       [512]
  then batched W3/FC head over the 32 samples.

  All index gathers are expressed as matmuls against one-hot matrices so the
  kernel uses NO indirect DMA:
    scatter(F)       = S_b^T @ F with S_b[j, n] = (sid[j] == n)  (on-device iota)
    An[:, nbr-cols]  = An @ E,  E one-hot neighbor columns       (host 0/1)
    An[g, nbr] terms = (E^T @ An @ G1h) * Msk                    (host 0/1)
  E/G1h/Msk encode only index structure (no float math on host).
"""
import os
import sys

if "/opt/trn_rl_repo" not in sys.path:
    sys.path.insert(0, "/opt/trn_rl_repo")

import numpy as np

import concourse.bass as bass
import concourse.mybir as mybir
import concourse.tile as tile
from concourse import bacc, bass_utils
from concourse.masks import make_identity

N_CORES = 8
BATCH, NODE, SEQ, FEAT = 256, 256, 128, 63   # FEAT = feature_num - 1
H1, H2, H3, FC = 512, 512, 1024, 512
BN_EPS = 1e-5
LEAKY = 0.01

F32 = mybir.dt.float32
BF16 = mybir.dt.bfloat16
I32 = mybir.dt.int32
AX = mybir.AxisListType
OP = mybir.AluOpType
ACTF = mybir.ActivationFunctionType


def build_nc(S: int, R: int):
    """Build the SPMD kernel for S samples per core, R neighbor slots."""
    COLS = S * R
    assert COLS % 128 == 0
    nc = bacc.Bacc("TRN2", target_bir_lowering=False, debug=False,
                   num_devices=N_CORES)

    x_d = nc.dram_tensor("x", [S, SEQ, FEAT + 2], F32, kind="ExternalInput").ap()
    adj_d = nc.dram_tensor("adj_mat", [NODE, NODE], F32, kind="ExternalInput").ap()
    W1b1_d = nc.dram_tensor("W1b1", [96, H1], F32, kind="ExternalInput").ap()
    W2_d = nc.dram_tensor("W2", [H1, H2], F32, kind="ExternalInput").ap()
    b2_d = nc.dram_tensor("b2", [H2], F32, kind="ExternalInput").ap()
    W3_d = nc.dram_tensor("W3", [H2, H3], F32, kind="ExternalInput").ap()
    b3_d = nc.dram_tensor("b3", [H3], F32, kind="ExternalInput").ap()
    fcW1_d = nc.dram_tensor("fcW1", [H3, FC], F32, kind="ExternalInput").ap()
    fcb1_d = nc.dram_tensor("fcb1", [FC], F32, kind="ExternalInput").ap()
    gamma_d = nc.dram_tensor("gamma", [FC], F32, kind="ExternalInput").ap()
    beta_d = nc.dram_tensor("beta", [FC], F32, kind="ExternalInput").ap()
    fcW2_d = nc.dram_tensor("fcW2", [FC, 1], F32, kind="ExternalInput").ap()
    fcb2_d = nc.dram_tensor("fcb2", [1], F32, kind="ExternalInput").ap()
    E_d = nc.dram_tensor("E", [NODE, COLS], F32, kind="ExternalInput").ap()
    G1h_d = nc.dram_tensor("G1h", [NODE, S], F32, kind="ExternalInput").ap()
    Msk_d = nc.dram_tensor("Msk", [COLS, S], F32, kind="ExternalInput").ap()
    out_d = nc.dram_tensor("out", [S, 1], F32, kind="ExternalOutput").ap()

    with tile.TileContext(nc) as tc:
        _body(nc, tc, S, R, x_d, adj_d, W1b1_d, W2_d, b2_d, W3_d, b3_d,
              fcW1_d, fcb1_d, gamma_d, beta_d, fcW2_d, fcb2_d,
              E_d, G1h_d, Msk_d, out_d)
    nc.compile()
    return nc


def _body(nc, tc, S, R, x_d, adj_d, W1b1_d, W2_d, b2_d, W3_d, b3_d,
          fcW1_d, fcb1_d, gamma_d, beta_d, fcW2_d, fcb2_d,
          E_d, G1h_d, Msk_d, out_d):
    COLS = S * R
    NCH = COLS // 128
    stage = int(os.environ.get("BISECT_STAGE", "0"))

    def stage_out(ap2d):
        # dump a [1, S] row to the output for bisection
        nc.sync.dma_start(out_d.rearrange("b o -> o b"), ap2d)

    with tc.tile_pool(name="const", bufs=1) as cp, \
         tc.tile_pool(name="dram", bufs=1, space="DRAM") as dp:

        # ---------------- constants & weights --------------------------------
        ident = cp.tile([128, 128], F32)
        make_identity(nc, ident[:])
        identb = cp.tile([128, 128], BF16)
        nc.vector.tensor_copy(identb[:], ident[:])
        iotaI = cp.tile([128, NODE], I32)
        nc.gpsimd.iota(iotaI[:], pattern=[[1, NODE]], base=0, channel_multiplier=0)
        iotaF = cp.tile([128, NODE], F32)
        nc.vector.tensor_copy(iotaF[:], iotaI[:])
        onesrow = cp.tile([1, 128], BF16)
        nc.gpsimd.memset(onesrow[:], 1.0)

        W1b1sb = cp.tile([96, H1], BF16)
        nc.gpsimd.dma_start(W1b1sb[:], W1b1_d[:])
        W2sb = cp.tile([128, 4, H2], BF16)
        nc.gpsimd.dma_start(W2sb[:], W2_d.rearrange("(k p) f -> p k f", p=128))
        b2row = cp.tile([1, H2], BF16)
        nc.gpsimd.dma_start(b2row[:], b2_d[None, :])
        W3sb = cp.tile([128, 4, H3], BF16)
        nc.gpsimd.dma_start(W3sb[:], W3_d.rearrange("(k p) h -> p k h", p=128))
        b3c = cp.tile([128, 8], F32)
        nc.sync.dma_start(b3c[:], b3_d.rearrange("(m p) -> p m", p=128))
        fcW1sb = cp.tile([128, 8, FC], BF16)
        nc.gpsimd.dma_start(fcW1sb[:], fcW1_d.rearrange("(k p) f -> p k f", p=128))
        fcb1c = cp.tile([128, 4], F32)
        nc.sync.dma_start(fcb1c[:], fcb1_d.rearrange("(m p) -> p m", p=128))
        gammac = cp.tile([128, 4], F32)
        nc.sync.dma_start(gammac[:], gamma_d.rearrange("(m p) -> p m", p=128))
        betac = cp.tile([128, 4], F32)
        nc.sync.dma_start(betac[:], beta_d.rearrange("(m p) -> p m", p=128))
        fcW2sb = cp.tile([128, 4], BF16)
        nc.gpsimd.dma_start(fcW2sb[:], fcW2_d.rearrange("(c p) o -> p (c o)", p=128))
        fcb2t = cp.tile([1, 1], F32)
        nc.sync.dma_start(fcb2t[:], fcb2_d[None, :])

        Esb = cp.tile([128, 2, COLS], BF16)
        nc.gpsimd.dma_start(Esb[:], E_d.rearrange("(k p) c -> p k c", p=128))
        G1sb = cp.tile([128, 2, S], BF16)
        nc.gpsimd.dma_start(G1sb[:], G1h_d.rearrange("(k p) b -> p k b", p=128))
        Msksb = cp.tile([128, NCH, S], BF16)
        nc.gpsimd.dma_start(Msksb[:], Msk_d.rearrange("(j p) b -> p j b", p=128))
        Fall = cp.tile([128, S, FEAT + 2], F32)   # [seq, sample, feat]
        nc.gpsimd.dma_start(Fall[:], x_d.rearrange("b j f -> j b f"))

        # ---------------- adjacency normalization ----------------------------
        # An = diag(dis) (A + I) diag(dis),  dis = 1/sqrt(rowsum(A) + 1)
        A0 = cp.tile([128, 2, NODE], F32)
        nc.sync.dma_start(A0[:], adj_d.rearrange("(c p) n -> p c n", p=128))
        rs = cp.tile([128, 2], F32)
        for c in range(2):
            nc.vector.tensor_reduce(rs[:, c:c + 1], A0[:, c, :], axis=AX.X, op=OP.add)
        sq = cp.tile([128, 2], F32)
        nc.scalar.activation(sq[:], rs[:], ACTF.Sqrt, bias=1.0)
        dis = cp.tile([128, 2], F32)
        nc.vector.reciprocal(dis[:], sq[:])
        dis2 = cp.tile([128, 2], F32)
        nc.vector.tensor_tensor(dis2[:], dis[:], dis[:], op=OP.mult)
        Csc = cp.tile([128, 2, NODE], F32)
        for c in range(2):
            nc.vector.tensor_scalar_mul(Csc[:, c, :], A0[:, c, :], dis[:, c:c + 1])
        Anb = cp.tile([128, 2, NODE], BF16)
        with tc.tile_pool(name="psPro", bufs=2, space="PSUM") as psP:
            for cd in range(2):
                for cs in range(2):
                    pT = psP.tile([128, 128], F32, tag="tr")
                    nc.tensor.transpose(pT[:], Csc[:, cs, 128 * cd:128 * (cd + 1)],
                                        ident[:])
                    nc.scalar.activation(Anb[:, cd, 128 * cs:128 * (cs + 1)], pT[:],
                                         ACTF.Copy, scale=dis[:, cd:cd + 1])
        diagb = cp.tile([128, 2, NODE], BF16)
        for c in range(2):
            nc.gpsimd.affine_select(
                out=diagb[:, c, :], in_=dis2[:, c:c + 1].to_broadcast([128, NODE]),
                pattern=[[-1, NODE]], compare_op=OP.is_equal, fill=0.0,
                base=128 * c, channel_multiplier=1)
            nc.vector.tensor_tensor(Anb[:, c, :], Anb[:, c, :], diagb[:, c, :],
                                    op=OP.add)
        if stage == 1:
            stage_out(Anb[0:1, 0, 0:S])
            return

        # ---------------- neighbor-column gathers as matmuls -----------------
        # AnNbrT[:, k, c] = An[128k+p, nbr(c)] = (An @ E) chunk
        AnNbrT = cp.tile([128, 2, COLS], BF16)
        with tc.tile_pool(name="psE", bufs=2, space="PSUM") as psE:
            for m in range(2):
                for cb in range(0, COLS, 512):
                    w = min(512, COLS - cb)
                    pE = psE.tile([128, 512], F32, tag="e")
                    for k in range(2):
                        nc.tensor.matmul(pE[:, 0:w],
                                         lhsT=Anb[:, k, 128 * m:128 * (m + 1)],
                                         rhs=Esb[:, k, cb:cb + w],
                                         start=(k == 0), stop=(k == 1))
                    if (m * 3 + cb // 512) % 2 == 0:
                        nc.scalar.activation(AnNbrT[:, m, cb:cb + w], pE[:, 0:w], ACTF.Copy)
                    else:
                        nc.vector.tensor_copy(AnNbrT[:, m, cb:cb + w], pE[:, 0:w])

            # AnG1h = An @ G1h  [256, S];  WSel = (E^T @ AnG1h) * Msk
            AnG1sb = cp.tile([128, 2, S], BF16)
            pG = psE.tile([128, 2, S], F32, tag="g")
            for m in range(2):
                for k in range(2):
                    nc.tensor.matmul(pG[:, m, :],
                                     lhsT=Anb[:, k, 128 * m:128 * (m + 1)],
                                     rhs=G1sb[:, k, :],
                                     start=(k == 0), stop=(k == 1))
            nc.vector.tensor_copy(AnG1sb[:], pG[:])
            WSel = cp.tile([128, NCH, S], BF16)
            for j in range(NCH):
                pW = psE.tile([128, S], F32, tag="w")
                for k in range(2):
                    nc.tensor.matmul(pW[:], lhsT=Esb[:, k, 128 * j:128 * (j + 1)],
                                     rhs=AnG1sb[:, k, :],
                                     start=(k == 0), stop=(k == 1))
                nc.vector.tensor_tensor(WSel[:, j, :], pW[:], Msksb[:, j, :],
                                        op=OP.mult)
        if stage == 2:
            stage_out(AnNbrT[0:1, 0, 0:S])
            return

        # ---------------- per-sample pipeline --------------------------------
        Z2T_all = cp.tile([128, 4, COLS], BF16)
        with tc.tile_pool(name="wl", bufs=2) as wl, \
             tc.tile_pool(name="psX0", bufs=1, space="PSUM") as psX0, \
             tc.tile_pool(name="psY", bufs=1, space="PSUM") as psY, \
             tc.tile_pool(name="psX1", bufs=2, space="PSUM") as psX1, \
             tc.tile_pool(name="psZ", bufs=1, space="PSUM") as psZ, \
             tc.tile_pool(name="psT", bufs=1, space="PSUM") as psT:
            for b in range(S):
                Fb = wl.tile([128, FEAT + 1], BF16, tag="Fb")
                nc.gpsimd.tensor_copy(Fb[:, 0:FEAT], Fall[:, b, 0:FEAT])
                nc.gpsimd.memset(Fb[:, FEAT:FEAT + 1], 0.0)
                Sb = wl.tile([128, NODE], BF16, tag="Sb")
                nc.gpsimd.tensor_scalar(
                    out=Sb[:], in0=iotaF[:], scalar1=Fall[:, b, FEAT:FEAT + 1],
                    scalar2=None, op0=OP.is_equal)
                # scatter: X0 = S_b^T @ F  [256, 63] node-major
                pX0 = psX0.tile([128, 2, FEAT + 1], F32, tag="x0")
                for k in range(2):
                    nc.tensor.matmul(pX0[:, k, :], lhsT=Sb[:, 128 * k:128 * (k + 1)],
                                     rhs=Fb[:], start=True, stop=True)
                X0sb = wl.tile([128, 2, FEAT + 1], BF16, tag="X0")
                nc.vector.tensor_copy(X0sb[:], pX0[:])
                # Y1T = X0^T @ An  [63, 256] feature-major
                pY1 = psY.tile([64, NODE], F32, tag="y1")
                for k in range(2):
                    nc.tensor.matmul(pY1[:], lhsT=X0sb[:, k, :],
                                     rhs=Anb[:, k, :], start=(k == 0), stop=(k == 1))
                Y1aug = wl.tile([96, NODE], BF16, tag="Y1aug")
                nc.scalar.activation(Y1aug[0:64, :], pY1[:], ACTF.Copy)
                nc.gpsimd.memset(Y1aug[64:96, :], 0.0)
                nc.gpsimd.memset(Y1aug[64:65, :], 1.0)
                if stage == 3:
                    if b == S - 1:
                        stage_out(Y1aug[0:1, 0:S])
                    continue
                # X1 = relu(Y1 @ W1 + b1)  [256, 512] node-major
                pX1 = psX1.tile([128, 2, H1], F32, tag="x1")
                for t in range(2):
                    nc.tensor.matmul(pX1[:, t, :],
                                     lhsT=Y1aug[:, 128 * t:128 * (t + 1)],
                                     rhs=W1b1sb[:], start=True, stop=True)
                X1sb = wl.tile([128, 2, H1], BF16, tag="X1")
                nc.scalar.activation(X1sb[:], pX1[:], ACTF.Relu)
                if stage == 4:
                    if b == S - 1:
                        stage_out(X1sb[0:1, 0, 0:S])
                    continue
                # Z2S = An[nbr,:] @ X1  [R, 512]
                pZ2 = psZ.tile([R, H2], F32, tag="z2")
                for k in range(2):
                    nc.tensor.matmul(pZ2[:], lhsT=AnNbrT[:, k, R * b:R * b + R],
                                     rhs=X1sb[:, k, :], start=(k == 0), stop=(k == 1))
                Z2Sb = wl.tile([R, H2], BF16, tag="Z2S")
                nc.vector.tensor_copy(Z2Sb[:], pZ2[:])
                # transpose to feature-major Z2T columns
                pZT = psT.tile([128, 4, R], BF16, tag="zt")
                for m in range(4):
                    nc.tensor.transpose(pZT[:, m, :], Z2Sb[:, 128 * m:128 * (m + 1)],
                                        identb[0:R, 0:R])
                nc.vector.tensor_copy(Z2T_all[:, :, R * b:R * b + R], pZT[:])

        if stage in (3, 4):
            return
        if stage == 5:
            stage_out(Z2T_all[0:1, 0, 0:S])
            return

        # ---------------- batched W2 + weighted reduce -----------------------
        X2S_all = cp.tile([128, NCH, H2], BF16)
        with tc.tile_pool(name="psW2", bufs=3, space="PSUM") as psW2:
            for j in range(NCH):
                pW2 = psW2.tile([128, H2], F32, tag="w2")
                nc.tensor.matmul(pW2[:], lhsT=onesrow[:], rhs=b2row[:],
                                 start=True, stop=False)
                for k in range(4):
                    nc.tensor.matmul(pW2[:], lhsT=Z2T_all[:, k, 128 * j:128 * (j + 1)],
                                     rhs=W2sb[:, k, :], start=False, stop=(k == 3))
                if j % 2 == 0:
                    nc.scalar.activation(X2S_all[:, j, :], pW2[:], ACTF.Relu)
                else:
                    nc.vector.tensor_scalar_max(X2S_all[:, j, :], pW2[:], 0.0)
            if stage == 6:
                stage_out(X2S_all[0:1, 0, 0:S])
                return

            # R = WSel^T @ X2S  [S, 512]
            pR = psW2.tile([S, H2], F32, tag="r")
            for j in range(NCH):
                nc.tensor.matmul(pR[:], lhsT=WSel[:, j, :], rhs=X2S_all[:, j, :],
                                 start=(j == 0), stop=(j == NCH - 1))
            Rb = cp.tile([S, H2], BF16)
            nc.scalar.activation(Rb[:], pR[:], ACTF.Copy)

        with tc.tile_pool(name="psEnd", bufs=2, space="PSUM") as psEnd:
            # RT [128, 4, S] feature-major
            pRT = psEnd.tile([128, 4, S], BF16, tag="rt")
            for m in range(4):
                nc.tensor.transpose(pRT[:, m, :], Rb[:, 128 * m:128 * (m + 1)],
                                    identb[0:S, 0:S])
            RTb = cp.tile([128, 4, S], BF16)
            nc.vector.tensor_copy(RTb[:], pRT[:])
            if stage == 7:
                stage_out(RTb[0:1, 0, 0:S])
                return

            # ---------------- G3 = relu(R @ W3 + b3), H = G3 @ fcW1 + fcb1 ---
            G3T = cp.tile([128, 8, S], BF16)
            for mb in range(8):
                pG3 = psEnd.tile([128, S], F32, tag="g3")
                for k in range(4):
                    nc.tensor.matmul(pG3[:], lhsT=W3sb[:, k, 128 * mb:128 * (mb + 1)],
                                     rhs=RTb[:, k, :], start=(k == 0), stop=(k == 3))
                nc.scalar.activation(G3T[:, mb, :], pG3[:], ACTF.Relu,
                                     bias=b3c[:, mb:mb + 1])
            HT = cp.tile([128, 4, S], F32)
            for m in range(4):
                pH = psEnd.tile([128, S], F32, tag="h")
                for k in range(8):
                    nc.tensor.matmul(pH[:], lhsT=fcW1sb[:, k, 128 * m:128 * (m + 1)],
                                     rhs=G3T[:, k, :], start=(k == 0), stop=(k == 7))
                nc.scalar.activation(HT[:, m, :], pH[:], ACTF.Identity,
                                     bias=fcb1c[:, m:m + 1])

            # ---------------- BatchNorm stats + AllReduce --------------------
            stats = cp.tile([128, 8], F32)
            sjunk = cp.tile([128, S], F32)
            for m in range(4):
                nc.vector.tensor_reduce(stats[:, m:m + 1], HT[:, m, :], axis=AX.X,
                                        op=OP.add)
                nc.scalar.activation(sjunk[:], HT[:, m, :], ACTF.Square,
                                     accum_out=stats[:, 4 + m:5 + m])
            if stage == 8:
                stage_out(stats[0:1, 0:S])
                return
            cc_in = dp.tile([128, 8], F32)
            cc_out = dp.tile([128, 8], F32)
            nc.sync.dma_start(cc_in[:], stats[:])
            nc.gpsimd.collective_compute(
                "AllReduce", OP.add, replica_groups=[list(range(N_CORES))],
                ins=[cc_in.opt()], outs=[cc_out.opt()])
            statsG = cp.tile([128, 8], F32)
            nc.sync.dma_start(statsG[:], cc_out[:])

            mean = cp.tile([128, 4], F32)
            ex2 = cp.tile([128, 4], F32)
            var = cp.tile([128, 4], F32)
            sd = cp.tile([128, 4], F32)
            rstd = cp.tile([128, 4], F32)
            scl = cp.tile([128, 4], F32)
            sft = cp.tile([128, 4], F32)
            nc.vector.tensor_scalar_mul(mean[:], statsG[:, 0:4], 1.0 / BATCH)
            nc.vector.tensor_scalar_mul(ex2[:], statsG[:, 4:8], 1.0 / BATCH)
            nc.vector.tensor_tensor(var[:], mean[:], mean[:], op=OP.mult)
            nc.vector.tensor_tensor(var[:], ex2[:], var[:], op=OP.subtract)
            epsc = cp.tile([128, 1], F32)
            nc.gpsimd.memset(epsc[:], BN_EPS)
            nc.scalar.activation(sd[:], var[:], ACTF.Sqrt, bias=epsc[:, 0:1])
            nc.vector.reciprocal(rstd[:], sd[:])
            nc.vector.tensor_tensor(scl[:], gammac[:], rstd[:], op=OP.mult)
            nc.vector.tensor_tensor(sft[:], mean[:], scl[:], op=OP.mult)
            nc.vector.tensor_tensor(sft[:], betac[:], sft[:], op=OP.subtract)

            # normalize + leaky relu, then FC2 + sigmoid
            Hl = cp.tile([128, 4, S], BF16)
            Hn = cp.tile([128, S], F32)
            for m in range(4):
                nc.scalar.activation(Hn[:], HT[:, m, :], ACTF.Identity,
                                     scale=scl[:, m:m + 1], bias=sft[:, m:m + 1])
                nc.vector.scalar_tensor_tensor(
                    out=Hl[:, m, :], in0=Hn[:], scalar=LEAKY, in1=Hn[:],
                    op0=OP.mult, op1=OP.max)
            pO = psEnd.tile([1, S], F32, tag="o")
            for c in range(4):
                nc.tensor.matmul(pO[:], lhsT=fcW2sb[:, c:c + 1], rhs=Hl[:, c, :],
                                 start=(c == 0), stop=(c == 3))
            outT = cp.tile([1, S], F32)
            nc.scalar.activation(outT[:], pO[:], ACTF.Sigmoid, bias=fcb2t[0:1, 0:1])
            nc.sync.dma_start(out_d.rearrange("b o -> o b"), outT[:])


_NC_CACHE = {}
_LAST_RESULT = None


def _get_nc(S: int, R: int):
    key = (S, R)
    if key not in _NC_CACHE:
        _NC_CACHE[key] = build_nc(S, R)
    return _NC_CACHE[key]


def _host_structure(x_slice, Abar_pattern, S, R):
    """Build one-hot index tensors (pure structure, no float math)."""
    COLS = S * R
    N = Abar_pattern.shape[0]
    g = x_slice[:, -1, -2].astype(np.int64)
    E = np.zeros((N, COLS), np.float32)
    G1h = np.zeros((N, S), np.float32)
    Msk = np.zeros((COLS, S), np.float32)
    for b in range(S):
        nbr = np.nonzero(Abar_pattern[g[b]])[0]
        cnt = len(nbr)
        E[nbr, R * b + np.arange(cnt)] = 1.0
        G1h[g[b], b] = 1.0
        Msk[R * b:R * b + cnt, b] = 1.0
    return E, G1h, Msk


def kernel(**inputs) -> np.ndarray:
    S = BATCH // N_CORES
    full_x = np.ascontiguousarray(inputs["x"], dtype=np.float32)
    adj = np.ascontiguousarray(inputs["adj_mat"], dtype=np.float32)
    Abar_pattern = (adj + np.eye(NODE, dtype=np.float32)) > 0
    max_nbr = int(Abar_pattern[full_x[:, -1, -2].astype(np.int64)].sum(1).max())
    R = 40 if max_nbr <= 40 else (48 if max_nbr <= 48 else 64)
    assert max_nbr <= 64, f"degree {max_nbr} exceeds kernel capacity"
    nc = _get_nc(S, R)

    shared = {}
    for k in ("adj_mat", "W2", "b2", "W3", "b3", "fcW1", "fcb1",
              "gamma", "beta", "fcW2", "fcb2"):
        shared[k] = np.ascontiguousarray(inputs[k], dtype=np.float32)
    W1b1 = np.zeros((96, H1), np.float32)
    W1b1[0:FEAT] = inputs["W1"]
    W1b1[64] = np.asarray(inputs["b1"])
    shared["W1b1"] = W1b1
    in_maps = []
    for c in range(N_CORES):
        m = dict(shared)
        xs = np.ascontiguousarray(full_x[c * S:(c + 1) * S])
        m["x"] = xs
        E, G1h, Msk = _host_structure(xs, Abar_pattern, S, R)
        m["E"], m["G1h"], m["Msk"] = E, G1h, Msk
        in_maps.append(m)
    res = bass_utils.run_bass_kernel_spmd(
        nc, in_maps, core_ids=list(range(N_CORES)))
    global _LAST_RESULT
    _LAST_RESULT = res
    out = np.concatenate([res.results[c]["out"] for c in range(N_CORES)], axis=0)
    return out.astype(np.float32)


if __name__ == "__main__":
    print("building...")
    nc = _get_nc(BATCH // N_CORES, 40)
    print("built ok")


# revision 12
# speedup vs baseline: 1.2786x; 1.2786x over previous
"""GCN2 Trainium2 kernel: 3-layer GCN + FC head with BatchNorm, 8-core data-parallel.

Self-contained: hardcodes shapes from the problem spec.
  x [256, 128, 65] f32, adj_mat [256, 256] f32, W1 [63, 512], b1 [512],
  W2 [512, 512], b2 [512], W3 [512, 1024], b3 [1024], fcW1 [1024, 512],
  fcb1 [512], gamma [512], beta [512], fcW2 [512, 1], fcb2 [1] -> out [256, 1]

Sharding: batch 256 -> 32 samples per core on 8 cores; weights/adj replicated.
BatchNorm batch stats all-reduced across cores (one small [128,8] AllReduce).

Algorithm (sparse tail):
  The output gather X3[b, g_b] means layers >= 2 are only needed at the
  neighbors of g_b (max degree+1 = R slots per sample). Per sample:
    X1 = relu(An @ scatter(F) @ W1 + b1)          dense [256, 512]
    Z2 = An[nbr(g), :] @ X1                       [R, 512]
    X2 = relu(Z2 @ W2 + b2)                       [R, 512]
    r  = An[g, nbr(g)] @ X2                       [512]
  then batched W3/FC head over the 32 samples.

  All index gathers are expressed as matmuls against one-hot matrices so the
  kernel uses NO indirect DMA:
    scatter(F)       = S_b^T @ F with S_b[j, n] = (sid[j] == n)  (on-device iota)
    An[:, nbr-cols]  = An @ E,  E one-hot neighbor columns       (host 0/1)
    An[g, nbr] terms = (E^T @ An @ G1h) * Msk                    (host 0/1)
  E/G1h/Msk encode only index structure (no float math on host).
"""
import os
import sys

if "/opt/trn_rl_repo" not in sys.path:
    sys.path.insert(0, "/opt/trn_rl_repo")

import numpy as np

import concourse.bass as bass
import concourse.mybir as mybir
import concourse.tile as tile
from concourse import bacc, bass_utils
from concourse.masks import make_identity

N_CORES = 8
BATCH, NODE, SEQ, FEAT = 256, 256, 128, 63   # FEAT = feature_num - 1
H1, H2, H3, FC = 512, 512, 1024, 512
BN_EPS = 1e-5
LEAKY = 0.01

F32 = mybir.dt.float32
BF16 = mybir.dt.bfloat16
I32 = mybir.dt.int32
AX = mybir.AxisListType
OP = mybir.AluOpType
ACTF = mybir.ActivationFunctionType


def build_nc(S: int, R: int):
    """Build the SPMD kernel for S samples per core, R neighbor slots."""
    COLS = S * R
    assert COLS % 128 == 0
    nc = bacc.Bacc("TRN2", target_bir_lowering=False, debug=False,
                   num_devices=N_CORES)

    x_d = nc.dram_tensor("x", [S, SEQ, FEAT + 2], F32, kind="ExternalInput").ap()
    adj_d = nc.dram_tensor("adj_mat", [NODE, NODE], F32, kind="ExternalInput").ap()
    W1b1_d = nc.dram_tensor("W1b1", [96, H1], F32, kind="ExternalInput").ap()
    W2_d = nc.dram_tensor("W2", [H1, H2], F32, kind="ExternalInput").ap()
    b2_d = nc.dram_tensor("b2", [H2], F32, kind="ExternalInput").ap()
    W3_d = nc.dram_tensor("W3", [H2, H3], F32, kind="ExternalInput").ap()
    b3_d = nc.dram_tensor("b3", [H3], F32, kind="ExternalInput").ap()
    fcW1_d = nc.dram_tensor("fcW1", [H3, FC], F32, kind="ExternalInput").ap()
    fcb1_d = nc.dram_tensor("fcb1", [FC], F32, kind="ExternalInput").ap()
    gamma_d = nc.dram_tensor("gamma", [FC], F32, kind="ExternalInput").ap()
    beta_d = nc.dram_tensor("beta", [FC], F32, kind="ExternalInput").ap()
    fcW2_d = nc.dram_tensor("fcW2", [FC, 1], F32, kind="ExternalInput").ap()
    fcb2_d = nc.dram_tensor("fcb2", [1], F32, kind="ExternalInput").ap()
    E_d = nc.dram_tensor("E", [NODE, COLS], F32, kind="ExternalInput").ap()
    G1h_d = nc.dram_tensor("G1h", [NODE, S], F32, kind="ExternalInput").ap()
    Msk_d = nc.dram_tensor("Msk", [COLS, S], F32, kind="ExternalInput").ap()
    out_d = nc.dram_tensor("out", [S, 1], F32, kind="ExternalOutput").ap()

    with tile.TileContext(nc) as tc:
        _body(nc, tc, S, R, x_d, adj_d, W1b1_d, W2_d, b2_d, W3_d, b3_d,
              fcW1_d, fcb1_d, gamma_d, beta_d, fcW2_d, fcb2_d,
              E_d, G1h_d, Msk_d, out_d)
    nc.compile()
    return nc


def _body(nc, tc, S, R, x_d, adj_d, W1b1_d, W2_d, b2_d, W3_d, b3_d,
          fcW1_d, fcb1_d, gamma_d, beta_d, fcW2_d, fcb2_d,
          E_d, G1h_d, Msk_d, out_d):
    COLS = S * R
    NCH = COLS // 128
    stage = int(os.environ.get("BISECT_STAGE", "0"))

    def stage_out(ap2d):
        # dump a [1, S] row to the output for bisection
        nc.sync.dma_start(out_d.rearrange("b o -> o b"), ap2d)

    with tc.tile_pool(name="const", bufs=1) as cp, \
         tc.tile_pool(name="dram", bufs=1, space="DRAM") as dp:

        # ---------------- constants & weights --------------------------------
        ident = cp.tile([128, 128], F32)
        make_identity(nc, ident[:])
        identb = cp.tile([128, 128], BF16)
        nc.vector.tensor_copy(identb[:], ident[:])
        iotaI = cp.tile([128, NODE], I32)
        nc.gpsimd.iota(iotaI[:], pattern=[[1, NODE]], base=0, channel_multiplier=0)
        iotaF = cp.tile([128, NODE], F32)
        nc.vector.tensor_copy(iotaF[:], iotaI[:])
        onesrow = cp.tile([1, 128], BF16)
        nc.gpsimd.memset(onesrow[:], 1.0)

        W1b1sb = cp.tile([96, H1], BF16)
        nc.gpsimd.dma_start(W1b1sb[:], W1b1_d[:])
        W2sb = cp.tile([128, 4, H2], BF16)
        nc.gpsimd.dma_start(W2sb[:], W2_d.rearrange("(k p) f -> p k f", p=128))
        b2row = cp.tile([1, H2], BF16)
        nc.gpsimd.dma_start(b2row[:], b2_d[None, :])
        W3sb = cp.tile([128, 4, H3], BF16)
        nc.gpsimd.dma_start(W3sb[:], W3_d.rearrange("(k p) h -> p k h", p=128))
        b3c = cp.tile([128, 8], F32)
        nc.sync.dma_start(b3c[:], b3_d.rearrange("(m p) -> p m", p=128))
        fcW1sb = cp.tile([128, 8, FC], BF16)
        nc.gpsimd.dma_start(fcW1sb[:], fcW1_d.rearrange("(k p) f -> p k f", p=128))
        fcb1c = cp.tile([128, 4], F32)
        nc.sync.dma_start(fcb1c[:], fcb1_d.rearrange("(m p) -> p m", p=128))
        gammac = cp.tile([128, 4], F32)
        nc.sync.dma_start(gammac[:], gamma_d.rearrange("(m p) -> p m", p=128))
        betac = cp.tile([128, 4], F32)
        nc.sync.dma_start(betac[:], beta_d.rearrange("(m p) -> p m", p=128))
        fcW2sb = cp.tile([128, 4], BF16)
        nc.gpsimd.dma_start(fcW2sb[:], fcW2_d.rearrange("(c p) o -> p (c o)", p=128))
        fcb2t = cp.tile([1, 1], F32)
        nc.sync.dma_start(fcb2t[:], fcb2_d[None, :])

        Esb = cp.tile([128, 2, COLS], BF16)
        nc.gpsimd.dma_start(Esb[:], E_d.rearrange("(k p) c -> p k c", p=128))
        G1sb = cp.tile([128, 2, S], BF16)
        nc.gpsimd.dma_start(G1sb[:], G1h_d.rearrange("(k p) b -> p k b", p=128))
        Msksb = cp.tile([128, NCH, S], BF16)
        nc.gpsimd.dma_start(Msksb[:], Msk_d.rearrange("(j p) b -> p j b", p=128))
        Fall = cp.tile([128, S, FEAT + 2], F32)   # [seq, sample, feat]
        nc.gpsimd.dma_start(Fall[:], x_d.rearrange("b j f -> j b f"))

        # ---------------- adjacency normalization ----------------------------
        # An = diag(dis) (A + I) diag(dis),  dis = 1/sqrt(rowsum(A) + 1)
        A0 = cp.tile([128, 2, NODE], F32)
        nc.sync.dma_start(A0[:], adj_d.rearrange("(c p) n -> p c n", p=128))
        rs = cp.tile([128, 2], F32)
        for c in range(2):
            nc.vector.tensor_reduce(rs[:, c:c + 1], A0[:, c, :], axis=AX.X, op=OP.add)
        sq = cp.tile([128, 2], F32)
        nc.scalar.activation(sq[:], rs[:], ACTF.Sqrt, bias=1.0)
        dis = cp.tile([128, 2], F32)
        nc.vector.reciprocal(dis[:], sq[:])
        dis2 = cp.tile([128, 2], F32)
        nc.vector.tensor_tensor(dis2[:], dis[:], dis[:], op=OP.mult)
        Csc = cp.tile([128, 2, NODE], F32)
        for c in range(2):
            nc.vector.tensor_scalar_mul(Csc[:, c, :], A0[:, c, :], dis[:, c:c + 1])
        Anb = cp.tile([128, 2, NODE], BF16)
        with tc.tile_pool(name="psPro", bufs=2, space="PSUM") as psP:
            for cd in range(2):
                for cs in range(2):
                    pT = psP.tile([128, 128], F32, tag="tr")
                    nc.tensor.transpose(pT[:], Csc[:, cs, 128 * cd:128 * (cd + 1)],
                                        ident[:])
                    nc.scalar.activation(Anb[:, cd, 128 * cs:128 * (cs + 1)], pT[:],
                                         ACTF.Copy, scale=dis[:, cd:cd + 1])
        diagb = cp.tile([128, 2, NODE], BF16)
        for c in range(2):
            nc.gpsimd.affine_select(
                out=diagb[:, c, :], in_=dis2[:, c:c + 1].to_broadcast([128, NODE]),
                pattern=[[-1, NODE]], compare_op=OP.is_equal, fill=0.0,
                base=128 * c, channel_multiplier=1)
            nc.vector.tensor_tensor(Anb[:, c, :], Anb[:, c, :], diagb[:, c, :],
                                    op=OP.add)
        if stage == 1:
            stage_out(Anb[0:1, 0, 0:S])
            return

        # ---------------- neighbor-column gathers as matmuls -----------------
        # AnNbrT[:, k, c] = An[128k+p, nbr(c)] = (An @ E) chunk
        AnNbrT = cp.tile([128, 2, COLS], BF16)
        with tc.tile_pool(name="psE", bufs=2, space="PSUM") as psE:
            for m in range(2):
                for cb in range(0, COLS, 512):
                    w = min(512, COLS - cb)
                    pE = psE.tile([128, 512], F32, tag="e")
                    for k in range(2):
                        nc.tensor.matmul(pE[:, 0:w],
                                         lhsT=Anb[:, k, 128 * m:128 * (m + 1)],
                                         rhs=Esb[:, k, cb:cb + w],
                                         start=(k == 0), stop=(k == 1))
                    if (m * 3 + cb // 512) % 2 == 0:
                        nc.scalar.activation(AnNbrT[:, m, cb:cb + w], pE[:, 0:w], ACTF.Copy)
                    else:
                        nc.vector.tensor_copy(AnNbrT[:, m, cb:cb + w], pE[:, 0:w])

            # AnG1h = An @ G1h  [256, S];  WSel = (E^T @ AnG1h) * Msk
            AnG1sb = cp.tile([128, 2, S], BF16)
            pG = psE.tile([128, 2, S], F32, tag="g")
            for m in range(2):
                for k in range(2):
                    nc.tensor.matmul(pG[:, m, :],
                                     lhsT=Anb[:, k, 128 * m:128 * (m + 1)],
                                     rhs=G1sb[:, k, :],
                                     start=(k == 0), stop=(k == 1))
            nc.vector.tensor_copy(AnG1sb[:], pG[:])
            WSel = cp.tile([128, NCH, S], BF16)
            for j in range(NCH):
                pW = psE.tile([128, S], F32, tag="w")
                for k in range(2):
                    nc.tensor.matmul(pW[:], lhsT=Esb[:, k, 128 * j:128 * (j + 1)],
                                     rhs=AnG1sb[:, k, :],
                                     start=(k == 0), stop=(k == 1))
                nc.vector.tensor_tensor(WSel[:, j, :], pW[:], Msksb[:, j, :],
                                        op=OP.mult)
        if stage == 2:
            stage_out(AnNbrT[0:1, 0, 0:S])
            return

        # ---------------- per-sample pipeline --------------------------------
        Z2T_all = cp.tile([128, 4, COLS], BF16)
        FbBufs = []
        Y1Bufs = []
        for i in range(2):
            fb = cp.tile([128, FEAT + 1], BF16, tag=f"Fb{i}")
            nc.gpsimd.memset(fb[:, FEAT:FEAT + 1], 0.0)
            FbBufs.append(fb)
            y1 = cp.tile([96, NODE], BF16, tag=f"Y1aug{i}")
            nc.gpsimd.memset(y1[64:96, :], 0.0)
            nc.gpsimd.memset(y1[64:65, :], 1.0)
            Y1Bufs.append(y1)
        with tc.tile_pool(name="wl", bufs=2) as wl, \
             tc.tile_pool(name="psX0", bufs=1, space="PSUM") as psX0, \
             tc.tile_pool(name="psY", bufs=1, space="PSUM") as psY, \
             tc.tile_pool(name="psX1", bufs=2, space="PSUM") as psX1, \
             tc.tile_pool(name="psZ", bufs=1, space="PSUM") as psZ, \
             tc.tile_pool(name="psT", bufs=1, space="PSUM") as psT:
            for b in range(S):
                Fb = FbBufs[b % 2]
                nc.gpsimd.tensor_copy(Fb[:, 0:FEAT], Fall[:, b, 0:FEAT])
                Sb = wl.tile([128, NODE], BF16, tag="Sb")
                nc.vector.tensor_scalar(
                    out=Sb[:], in0=iotaF[:], scalar1=Fall[:, b, FEAT:FEAT + 1],
                    scalar2=None, op0=OP.is_equal)
                # scatter: X0 = S_b^T @ F  [256, 63] node-major
                pX0 = psX0.tile([128, 2, FEAT + 1], F32, tag="x0")
                for k in range(2):
                    nc.tensor.matmul(pX0[:, k, :], lhsT=Sb[:, 128 * k:128 * (k + 1)],
                                     rhs=Fb[:], start=True, stop=True)
                X0sb = wl.tile([128, 2, FEAT + 1], BF16, tag="X0")
                nc.vector.tensor_copy(X0sb[:], pX0[:])
                # Y1T = X0^T @ An  [63, 256] feature-major
                pY1 = psY.tile([64, NODE], F32, tag="y1")
                for k in range(2):
                    nc.tensor.matmul(pY1[:], lhsT=X0sb[:, k, :],
                                     rhs=Anb[:, k, :], start=(k == 0), stop=(k == 1))
                Y1aug = Y1Bufs[b % 2]
                nc.scalar.activation(Y1aug[0:64, :], pY1[:], ACTF.Copy)
                if stage == 3:
                    if b == S - 1:
                        stage_out(Y1aug[0:1, 0:S])
                    continue
                # X1 = relu(Y1 @ W1 + b1)  [256, 512] node-major
                pX1 = psX1.tile([128, 2, H1], F32, tag="x1")
                for t in range(2):
                    nc.tensor.matmul(pX1[:, t, :],
                                     lhsT=Y1aug[:, 128 * t:128 * (t + 1)],
                                     rhs=W1b1sb[:], start=True, stop=True)
                X1sb = wl.tile([128, 2, H1], BF16, tag="X1")
                nc.scalar.activation(X1sb[:], pX1[:], ACTF.Relu)
                if stage == 4:
                    if b == S - 1:
                        stage_out(X1sb[0:1, 0, 0:S])
                    continue
                # Z2S = An[nbr,:] @ X1  [R, 512]
                pZ2 = psZ.tile([R, H2], F32, tag="z2")
                for k in range(2):
                    nc.tensor.matmul(pZ2[:], lhsT=AnNbrT[:, k, R * b:R * b + R],
                                     rhs=X1sb[:, k, :], start=(k == 0), stop=(k == 1))
                Z2Sb = wl.tile([R, H2], BF16, tag="Z2S")
                nc.vector.tensor_copy(Z2Sb[:], pZ2[:])
                # transpose to feature-major Z2T columns
                pZT = psT.tile([128, 4, R], BF16, tag="zt")
                for m in range(4):
                    nc.tensor.transpose(pZT[:, m, :], Z2Sb[:, 128 * m:128 * (m + 1)],
                                        identb[0:R, 0:R])
                nc.vector.tensor_copy(Z2T_all[:, :, R * b:R * b + R], pZT[:])

        if stage in (3, 4):
            return
        if stage == 5:
            stage_out(Z2T_all[0:1, 0, 0:S])
            return

        # ---------------- batched W2 + weighted reduce -----------------------
        X2S_all = cp.tile([128, NCH, H2], BF16)
        with tc.tile_pool(name="psW2", bufs=3, space="PSUM") as psW2:
            for j in range(NCH):
                pW2 = psW2.tile([128, H2], F32, tag="w2")
                nc.tensor.matmul(pW2[:], lhsT=onesrow[:], rhs=b2row[:],
                                 start=True, stop=False)
                for k in range(4):
                    nc.tensor.matmul(pW2[:], lhsT=Z2T_all[:, k, 128 * j:128 * (j + 1)],
                                     rhs=W2sb[:, k, :], start=False, stop=(k == 3))
                if j % 2 == 0:
                    nc.scalar.activation(X2S_all[:, j, :], pW2[:], ACTF.Relu)
                else:
                    nc.vector.tensor_scalar_max(X2S_all[:, j, :], pW2[:], 0.0)
            if stage == 6:
                stage_out(X2S_all[0:1, 0, 0:S])
                return

            # R = WSel^T @ X2S  [S, 512]
            pR = psW2.tile([S, H2], F32, tag="r")
            for j in range(NCH):
                nc.tensor.matmul(pR[:], lhsT=WSel[:, j, :], rhs=X2S_all[:, j, :],
                                 start=(j == 0), stop=(j == NCH - 1))
            Rb = cp.tile([S, H2], BF16)
            nc.scalar.activation(Rb[:], pR[:], ACTF.Copy)

        with tc.tile_pool(name="psEnd", bufs=2, space="PSUM") as psEnd:
            # RT [128, 4, S] feature-major
            pRT = psEnd.tile([128, 4, S], BF16, tag="rt")
            for m in range(4):
                nc.tensor.transpose(pRT[:, m, :], Rb[:, 128 * m:128 * (m + 1)],
                                    identb[0:S, 0:S])
            RTb = cp.tile([128, 4, S], BF16)
            nc.vector.tensor_copy(RTb[:], pRT[:])
            if stage == 7:
                stage_out(RTb[0:1, 0, 0:S])
                return

            # ---------------- G3 = relu(R @ W3 + b3), H = G3 @ fcW1 + fcb1 ---
            G3T = cp.tile([128, 8, S], BF16)
            for mb in range(8):
                pG3 = psEnd.tile([128, S], F32, tag="g3")
                for k in range(4):
                    nc.tensor.matmul(pG3[:], lhsT=W3sb[:, k, 128 * mb:128 * (mb + 1)],
                                     rhs=RTb[:, k, :], start=(k == 0), stop=(k == 3))
                nc.scalar.activation(G3T[:, mb, :], pG3[:], ACTF.Relu,
                                     bias=b3c[:, mb:mb + 1])
            HT = cp.tile([128, 4, S], F32)
            for m in range(4):
                pH = psEnd.tile([128, S], F32, tag="h")
                for k in range(8):
                    nc.tensor.matmul(pH[:], lhsT=fcW1sb[:, k, 128 * m:128 * (m + 1)],
                                     rhs=G3T[:, k, :], start=(k == 0), stop=(k == 7))
                nc.scalar.activation(HT[:, m, :], pH[:], ACTF.Identity,
                                     bias=fcb1c[:, m:m + 1])

            # ---------------- BatchNorm stats + AllReduce --------------------
            stats = cp.tile([128, 8], F32)
            sjunk = cp.tile([128, S], F32)
            for m in range(4):
                nc.vector.tensor_reduce(stats[:, m:m + 1], HT[:, m, :], axis=AX.X,
                                        op=OP.add)
                nc.scalar.activation(sjunk[:], HT[:, m, :], ACTF.Square,
                                     accum_out=stats[:, 4 + m:5 + m])
            if stage == 8:
                stage_out(stats[0:1, 0:S])
                return
            cc_in = dp.tile([128, 8], F32)
            cc_out = dp.tile([128, 8], F32)
            nc.sync.dma_start(cc_in[:], stats[:])
            nc.gpsimd.collective_compute(
                "AllReduce", OP.add, replica_groups=[list(range(N_CORES))],
                ins=[cc_in.opt()], outs=[cc_out.opt()])
            statsG = cp.tile([128, 8], F32)
            nc.sync.dma_start(statsG[:], cc_out[:])

            mean = cp.tile([128, 4], F32)
            ex2 = cp.tile([128, 4], F32)
            var = cp.tile([128, 4], F32)
            sd = cp.tile([128, 4], F32)
            rstd = cp.tile([128, 4], F32)
            scl = cp.tile([128, 4], F32)
            sft = cp.tile([128, 4], F32)
            nc.vector.tensor_scalar_mul(mean[:], statsG[:, 0:4], 1.0 / BATCH)
            nc.vector.tensor_scalar_mul(ex2[:], statsG[:, 4:8], 1.0 / BATCH)
            nc.vector.tensor_tensor(var[:], mean[:], mean[:], op=OP.mult)
            nc.vector.tensor_tensor(var[:], ex2[:], var[:], op=OP.subtract)
            epsc = cp.tile([128, 1], F32)
            nc.gpsimd.memset(epsc[:], BN_EPS)
            nc.scalar.activation(sd[:], var[:], ACTF.Sqrt, bias=epsc[:, 0:1])
            nc.vector.reciprocal(rstd[:], sd[:])
            nc.vector.tensor_tensor(scl[:], gammac[:], rstd[:], op=OP.mult)
            nc.vector.tensor_tensor(sft[:], mean[:], scl[:], op=OP.mult)
            nc.vector.tensor_tensor(sft[:], betac[:], sft[:], op=OP.subtract)

            # normalize + leaky relu, then FC2 + sigmoid
            Hl = cp.tile([128, 4, S], BF16)
            Hn = cp.tile([128, S], F32)
            for m in range(4):
                nc.scalar.activation(Hn[:], HT[:, m, :], ACTF.Identity,
                                     scale=scl[:, m:m + 1], bias=sft[:, m:m + 1])
                nc.vector.scalar_tensor_tensor(
                    out=Hl[:, m, :], in0=Hn[:], scalar=LEAKY, in1=Hn[:],
                    op0=OP.mult, op1=OP.max)
            pO = psEnd.tile([1, S], F32, tag="o")
            for c in range(4):
                nc.tensor.matmul(pO[:], lhsT=fcW2sb[:, c:c + 1], rhs=Hl[:, c, :],
                                 start=(c == 0), stop=(c == 3))
            outT = cp.tile([1, S], F32)
            nc.scalar.activation(outT[:], pO[:], ACTF.Sigmoid, bias=fcb2t[0:1, 0:1])
            nc.sync.dma_start(out_d.rearrange("b o -> o b"), outT[:])


_NC_CACHE = {}
_LAST_RESULT = None


def _get_nc(S: int, R: int):
    key = (S, R)
    if key not in _NC_CACHE:
        _NC_CACHE[key] = build_nc(S, R)
    return _NC_CACHE[key]


def _host_structure(x_slice, Abar_pattern, S, R):
    """Build one-hot index tensors (pure structure, no float math)."""
    COLS = S * R
    N = Abar_pattern.shape[0]
    g = x_slice[:, -1, -2].astype(np.int64)
    E = np.zeros((N, COLS), np.float32)
    G1h = np.zeros((N, S), np.float32)
    Msk = np.zeros((COLS, S), np.float32)
    for b in range(S):
        nbr = np.nonzero(Abar_pattern[g[b]])[0]
        cnt = len(nbr)
        E[nbr, R * b + np.arange(cnt)] = 1.0
        G1h[g[b], b] = 1.0
        Msk[R * b:R * b + cnt, b] = 1.0
    return E, G1h, Msk


def kernel(**inputs) -> np.ndarray:
    S = BATCH // N_CORES
    full_x = np.ascontiguousarray(inputs["x"], dtype=np.float32)
    adj = np.ascontiguousarray(inputs["adj_mat"], dtype=np.float32)
    Abar_pattern = (adj + np.eye(NODE, dtype=np.float32)) > 0
    max_nbr = int(Abar_pattern[full_x[:, -1, -2].astype(np.int64)].sum(1).max())
    R = 40 if max_nbr <= 40 else (48 if max_nbr <= 48 else 64)
    assert max_nbr <= 64, f"degree {max_nbr} exceeds kernel capacity"
    nc = _get_nc(S, R)

    shared = {}
    for k in ("adj_mat", "W2", "b2", "W3", "b3", "fcW1", "fcb1",
              "gamma", "beta", "fcW2", "fcb2"):
        shared[k] = np.ascontiguousarray(inputs[k], dtype=np.float32)
    W1b1 = np.zeros((96, H1), np.float32)
    W1b1[0:FEAT] = inputs["W1"]
    W1b1[64] = np.asarray(inputs["b1"])
    shared["W1b1"] = W1b1
    in_maps = []
    for c in range(N_CORES):
        m = dict(shared)
        xs = np.ascontiguousarray(full_x[c * S:(c + 1) * S])
        m["x"] = xs
        E, G1h, Msk = _host_structure(xs, Abar_pattern, S, R)
        m["E"], m["G1h"], m["Msk"] = E, G1h, Msk
        in_maps.append(m)
    res = bass_utils.run_bass_kernel_spmd(
        nc, in_maps, core_ids=list(range(N_CORES)))
    global _LAST_RESULT
    _LAST_RESULT = res
    out = np.concatenate([res.results[c]["out"] for c in range(N_CORES)], axis=0)
    return out.astype(np.float32)


if __name__ == "__main__":
    print("building...")
    nc = _get_nc(BATCH // N_CORES, 40)
    print("built ok")


# revision 13
# speedup vs baseline: 1.5660x; 1.2248x over previous
"""GCN2 Trainium2 kernel: 3-layer GCN + FC head with BatchNorm, 8-core data-parallel.

Self-contained: hardcodes shapes from the problem spec.
  x [256, 128, 65] f32, adj_mat [256, 256] f32, W1 [63, 512], b1 [512],
  W2 [512, 512], b2 [512], W3 [512, 1024], b3 [1024], fcW1 [1024, 512],
  fcb1 [512], gamma [512], beta [512], fcW2 [512, 1], fcb2 [1] -> out [256, 1]

Sharding: batch 256 -> 32 samples per core on 8 cores; weights/adj replicated.
BatchNorm batch stats all-reduced across cores (one small [128,8] AllReduce).

Algorithm (sparse tail):
  The output gather X3[b, g_b] means layers >= 2 are only needed at the
  neighbors of g_b (max degree+1 = R slots per sample). Per sample:
    X1 = relu(An @ scatter(F) @ W1 + b1)          dense [256, 512]
    Z2 = An[nbr(g), :] @ X1                       [R, 512]
    X2 = relu(Z2 @ W2 + b2)                       [R, 512]
    r  = An[g, nbr(g)] @ X2                       [512]
  then batched W3/FC head over the 32 samples.

  All index gathers are expressed as matmuls against one-hot matrices so the
  kernel uses NO indirect DMA:
    scatter(F)       = S_b^T @ F with S_b[j, n] = (sid[j] == n)  (on-device iota)
    An[:, nbr-cols]  = An @ E,  E one-hot neighbor columns       (host 0/1)
    An[g, nbr] terms = (E^T @ An @ G1h) * Msk                    (host 0/1)
  E/G1h/Msk encode only index structure (no float math on host).
"""
import os
import sys

if "/opt/trn_rl_repo" not in sys.path:
    sys.path.insert(0, "/opt/trn_rl_repo")

import numpy as np

import concourse.bass as bass
import concourse.mybir as mybir
import concourse.tile as tile
from concourse import bacc, bass_utils
from concourse.masks import make_identity

N_CORES = 8
BATCH, NODE, SEQ, FEAT = 256, 256, 128, 63   # FEAT = feature_num - 1
H1, H2, H3, FC = 512, 512, 1024, 512
BN_EPS = 1e-5
LEAKY = 0.01

F32 = mybir.dt.float32
BF16 = mybir.dt.bfloat16
I32 = mybir.dt.int32
AX = mybir.AxisListType
OP = mybir.AluOpType
ACTF = mybir.ActivationFunctionType


def build_nc(S: int, R: int):
    """Build the SPMD kernel for S samples per core, R neighbor slots."""
    COLS = S * R
    assert COLS % 128 == 0
    nc = bacc.Bacc("TRN2", target_bir_lowering=False, debug=False,
                   num_devices=N_CORES)

    x_d = nc.dram_tensor("x", [S, SEQ, FEAT + 2], F32, kind="ExternalInput").ap()
    adj_d = nc.dram_tensor("adj_mat", [NODE, NODE], F32, kind="ExternalInput").ap()
    W1b1_d = nc.dram_tensor("W1b1", [96, H1], F32, kind="ExternalInput").ap()
    W2_d = nc.dram_tensor("W2", [H1, H2], F32, kind="ExternalInput").ap()
    b2_d = nc.dram_tensor("b2", [H2], F32, kind="ExternalInput").ap()
    W3_d = nc.dram_tensor("W3", [H2, H3], F32, kind="ExternalInput").ap()
    b3_d = nc.dram_tensor("b3", [H3], F32, kind="ExternalInput").ap()
    fcW1_d = nc.dram_tensor("fcW1", [H3, FC], F32, kind="ExternalInput").ap()
    fcb1_d = nc.dram_tensor("fcb1", [FC], F32, kind="ExternalInput").ap()
    gamma_d = nc.dram_tensor("gamma", [FC], F32, kind="ExternalInput").ap()
    beta_d = nc.dram_tensor("beta", [FC], F32, kind="ExternalInput").ap()
    fcW2_d = nc.dram_tensor("fcW2", [FC, 1], F32, kind="ExternalInput").ap()
    fcb2_d = nc.dram_tensor("fcb2", [1], F32, kind="ExternalInput").ap()
    E_d = nc.dram_tensor("E", [NODE, COLS], F32, kind="ExternalInput").ap()
    G1h_d = nc.dram_tensor("G1h", [NODE, S], F32, kind="ExternalInput").ap()
    Msk_d = nc.dram_tensor("Msk", [COLS, S], F32, kind="ExternalInput").ap()
    out_d = nc.dram_tensor("out", [S, 1], F32, kind="ExternalOutput").ap()

    with tile.TileContext(nc) as tc:
        _body(nc, tc, S, R, x_d, adj_d, W1b1_d, W2_d, b2_d, W3_d, b3_d,
              fcW1_d, fcb1_d, gamma_d, beta_d, fcW2_d, fcb2_d,
              E_d, G1h_d, Msk_d, out_d)
    nc.compile()
    return nc


def _body(nc, tc, S, R, x_d, adj_d, W1b1_d, W2_d, b2_d, W3_d, b3_d,
          fcW1_d, fcb1_d, gamma_d, beta_d, fcW2_d, fcb2_d,
          E_d, G1h_d, Msk_d, out_d):
    COLS = S * R
    NCH = COLS // 128
    stage = int(os.environ.get("BISECT_STAGE", "0"))

    def stage_out(ap2d):
        # dump a [1, S] row to the output for bisection
        nc.sync.dma_start(out_d.rearrange("b o -> o b"), ap2d)

    with tc.tile_pool(name="const", bufs=1) as cp, \
         tc.tile_pool(name="dram", bufs=1, space="DRAM") as dp:

        # ---------------- constants & weights --------------------------------
        ident = cp.tile([128, 128], F32)
        make_identity(nc, ident[:])
        identb = cp.tile([128, 128], BF16)
        nc.vector.tensor_copy(identb[:], ident[:])
        iotaI = cp.tile([128, NODE], I32)
        nc.gpsimd.iota(iotaI[:], pattern=[[1, NODE]], base=0, channel_multiplier=0)
        iotaF = cp.tile([128, NODE], F32)
        nc.vector.tensor_copy(iotaF[:], iotaI[:])
        onesrow = cp.tile([1, 128], BF16)
        nc.gpsimd.memset(onesrow[:], 1.0)

        W1b1sb = cp.tile([96, H1], BF16)
        nc.gpsimd.dma_start(W1b1sb[:], W1b1_d[:])
        W2sb = cp.tile([128, 4, H2], BF16)
        nc.gpsimd.dma_start(W2sb[:], W2_d.rearrange("(k p) f -> p k f", p=128))
        b2row = cp.tile([1, H2], BF16)
        nc.gpsimd.dma_start(b2row[:], b2_d[None, :])
        W3sb = cp.tile([128, 4, H3], BF16)
        nc.gpsimd.dma_start(W3sb[:], W3_d.rearrange("(k p) h -> p k h", p=128))
        b3c = cp.tile([128, 8], F32)
        nc.sync.dma_start(b3c[:], b3_d.rearrange("(m p) -> p m", p=128))
        fcW1sb = cp.tile([128, 8, FC], BF16)
        nc.gpsimd.dma_start(fcW1sb[:], fcW1_d.rearrange("(k p) f -> p k f", p=128))
        fcb1c = cp.tile([128, 4], F32)
        nc.sync.dma_start(fcb1c[:], fcb1_d.rearrange("(m p) -> p m", p=128))
        gammac = cp.tile([128, 4], F32)
        nc.sync.dma_start(gammac[:], gamma_d.rearrange("(m p) -> p m", p=128))
        betac = cp.tile([128, 4], F32)
        nc.sync.dma_start(betac[:], beta_d.rearrange("(m p) -> p m", p=128))
        fcW2sb = cp.tile([128, 4], BF16)
        nc.gpsimd.dma_start(fcW2sb[:], fcW2_d.rearrange("(c p) o -> p (c o)", p=128))
        fcb2t = cp.tile([1, 1], F32)
        nc.sync.dma_start(fcb2t[:], fcb2_d[None, :])

        Esb = cp.tile([128, 2, COLS], BF16)
        nc.gpsimd.dma_start(Esb[:], E_d.rearrange("(k p) c -> p k c", p=128))
        G1sb = cp.tile([128, 2, S], BF16)
        nc.gpsimd.dma_start(G1sb[:], G1h_d.rearrange("(k p) b -> p k b", p=128))
        Msksb = cp.tile([128, NCH, S], BF16)
        nc.gpsimd.dma_start(Msksb[:], Msk_d.rearrange("(j p) b -> p j b", p=128))
        Fall = cp.tile([128, S, FEAT + 2], F32)   # [seq, sample, feat]
        nc.gpsimd.dma_start(Fall[:], x_d.rearrange("b j f -> j b f"))

        # ---------------- adjacency normalization ----------------------------
        # An = diag(dis) (A + I) diag(dis),  dis = 1/sqrt(rowsum(A) + 1)
        A0 = cp.tile([128, 2, NODE], F32)
        nc.sync.dma_start(A0[:], adj_d.rearrange("(c p) n -> p c n", p=128))
        rs = cp.tile([128, 2], F32)
        for c in range(2):
            nc.vector.tensor_reduce(rs[:, c:c + 1], A0[:, c, :], axis=AX.X, op=OP.add)
        sq = cp.tile([128, 2], F32)
        nc.scalar.activation(sq[:], rs[:], ACTF.Sqrt, bias=1.0)
        dis = cp.tile([128, 2], F32)
        nc.vector.reciprocal(dis[:], sq[:])
        dis2 = cp.tile([128, 2], F32)
        nc.vector.tensor_tensor(dis2[:], dis[:], dis[:], op=OP.mult)
        Csc = cp.tile([128, 2, NODE], F32)
        for c in range(2):
            nc.vector.tensor_scalar_mul(Csc[:, c, :], A0[:, c, :], dis[:, c:c + 1])
        Anb = cp.tile([128, 2, NODE], BF16)
        with tc.tile_pool(name="psPro", bufs=2, space="PSUM") as psP:
            for cd in range(2):
                for cs in range(2):
                    pT = psP.tile([128, 128], F32, tag="tr")
                    nc.tensor.transpose(pT[:], Csc[:, cs, 128 * cd:128 * (cd + 1)],
                                        ident[:])
                    nc.scalar.activation(Anb[:, cd, 128 * cs:128 * (cs + 1)], pT[:],
                                         ACTF.Copy, scale=dis[:, cd:cd + 1])
        diagb = cp.tile([128, 2, NODE], BF16)
        for c in range(2):
            nc.gpsimd.affine_select(
                out=diagb[:, c, :], in_=dis2[:, c:c + 1].to_broadcast([128, NODE]),
                pattern=[[-1, NODE]], compare_op=OP.is_equal, fill=0.0,
                base=128 * c, channel_multiplier=1)
            nc.vector.tensor_tensor(Anb[:, c, :], Anb[:, c, :], diagb[:, c, :],
                                    op=OP.add)
        if stage == 1:
            stage_out(Anb[0:1, 0, 0:S])
            return

        # ---------------- neighbor-column gathers as matmuls -----------------
        # AnNbrT[:, k, c] = An[128k+p, nbr(c)] = (An @ E) chunk
        AnNbrT = cp.tile([128, 2, COLS], BF16)
        with tc.tile_pool(name="psE", bufs=2, space="PSUM") as psE:
            for m in range(2):
                for cb in range(0, COLS, 512):
                    w = min(512, COLS - cb)
                    pE = psE.tile([128, 512], F32, tag="e")
                    for k in range(2):
                        nc.tensor.matmul(pE[:, 0:w],
                                         lhsT=Anb[:, k, 128 * m:128 * (m + 1)],
                                         rhs=Esb[:, k, cb:cb + w],
                                         start=(k == 0), stop=(k == 1))
                    if (m * 3 + cb // 512) % 2 == 0:
                        nc.scalar.activation(AnNbrT[:, m, cb:cb + w], pE[:, 0:w], ACTF.Copy)
                    else:
                        nc.vector.tensor_copy(AnNbrT[:, m, cb:cb + w], pE[:, 0:w])

            # AnG1h = An @ G1h  [256, S];  WSel = (E^T @ AnG1h) * Msk
            AnG1sb = cp.tile([128, 2, S], BF16)
            pG = psE.tile([128, 2, S], F32, tag="g")
            for m in range(2):
                for k in range(2):
                    nc.tensor.matmul(pG[:, m, :],
                                     lhsT=Anb[:, k, 128 * m:128 * (m + 1)],
                                     rhs=G1sb[:, k, :],
                                     start=(k == 0), stop=(k == 1))
            nc.vector.tensor_copy(AnG1sb[:], pG[:])
            WSel = cp.tile([128, NCH, S], BF16)
            for j in range(NCH):
                pW = psE.tile([128, S], F32, tag="w")
                for k in range(2):
                    nc.tensor.matmul(pW[:], lhsT=Esb[:, k, 128 * j:128 * (j + 1)],
                                     rhs=AnG1sb[:, k, :],
                                     start=(k == 0), stop=(k == 1))
                nc.vector.tensor_tensor(WSel[:, j, :], pW[:], Msksb[:, j, :],
                                        op=OP.mult)
        if stage == 2:
            stage_out(AnNbrT[0:1, 0, 0:S])
            return

        # ---------------- per-sample pipeline --------------------------------
        Z2T_all = cp.tile([128, 4, COLS], BF16)
        FbBufs = []
        Y1Bufs = []
        for i in range(2):
            fb = cp.tile([128, FEAT + 1], BF16, tag=f"Fb{i}")
            nc.gpsimd.memset(fb[:, FEAT:FEAT + 1], 0.0)
            FbBufs.append(fb)
            y1 = cp.tile([96, NODE], BF16, tag=f"Y1aug{i}")
            nc.gpsimd.memset(y1[64:96, :], 0.0)
            nc.gpsimd.memset(y1[64:65, :], 1.0)
            Y1Bufs.append(y1)
        with tc.tile_pool(name="wl", bufs=2) as wl, \
             tc.tile_pool(name="psX0", bufs=2, space="PSUM") as psX0, \
             tc.tile_pool(name="psY", bufs=2, space="PSUM") as psY, \
             tc.tile_pool(name="psX1", bufs=1, space="PSUM") as psX1, \
             tc.tile_pool(name="psZ", bufs=1, space="PSUM") as psZ, \
             tc.tile_pool(name="psT", bufs=1, space="PSUM") as psT:
            # Software-pipelined across samples: stage s of sample b is
            # emitted at virtual step v = b + OFF[s], so each PSUM->SBUF
            # evacuation has a full step of other samples' matmuls to hide
            # its latency before its consumer runs.
            Sbs, X0s, X1s, Z2s = {}, {}, {}, {}
            pX0s, pY1s, pX1s, pZ2s, pZTs = {}, {}, {}, {}, {}

            def s0(b):   # input prep (Pool + DVE)
                Fb = FbBufs[b % 2]
                nc.gpsimd.tensor_copy(Fb[:, 0:FEAT], Fall[:, b, 0:FEAT])
                Sb = wl.tile([128, NODE], BF16, tag="Sb")
                nc.vector.tensor_scalar(
                    out=Sb[:], in0=iotaF[:], scalar1=Fall[:, b, FEAT:FEAT + 1],
                    scalar2=None, op0=OP.is_equal)
                Sbs[b] = Sb

            def s1(b):   # scatter: X0 = S_b^T @ F  [256, 64] node-major
                Fb, Sb = FbBufs[b % 2], Sbs.pop(b)
                pX0 = psX0.tile([128, 2, FEAT + 1], F32, tag="x0")
                for k in range(2):
                    nc.tensor.matmul(pX0[:, k, :], lhsT=Sb[:, 128 * k:128 * (k + 1)],
                                     rhs=Fb[:], start=True, stop=True)
                X0sb = wl.tile([128, 2, FEAT + 1], BF16, tag="X0")
                nc.vector.tensor_copy(X0sb[:], pX0[:])
                X0s[b] = X0sb

            def s2(b):   # Y1T = X0^T @ An  [64, 256] feature-major
                X0sb = X0s.pop(b)
                pY1 = psY.tile([64, NODE], F32, tag="y1")
                for k in range(2):
                    nc.tensor.matmul(pY1[:], lhsT=X0sb[:, k, :],
                                     rhs=Anb[:, k, :], start=(k == 0), stop=(k == 1))
                Y1aug = Y1Bufs[b % 2]
                nc.scalar.activation(Y1aug[0:64, :], pY1[:], ACTF.Copy)

            def s3(b):   # X1 = relu(Y1 @ W1 + b1)  [256, 512] node-major
                Y1aug = Y1Bufs[b % 2]
                pX1 = psX1.tile([128, 2, H1], F32, tag="x1")
                for t in range(2):
                    nc.tensor.matmul(pX1[:, t, :],
                                     lhsT=Y1aug[:, 128 * t:128 * (t + 1)],
                                     rhs=W1b1sb[:], start=True, stop=True)
                X1sb = wl.tile([128, 2, H1], BF16, tag="X1")
                nc.scalar.activation(X1sb[:], pX1[:], ACTF.Relu)
                X1s[b] = X1sb

            def s4(b):   # Z2S = An[nbr,:] @ X1  [R, 512]
                X1sb = X1s.pop(b)
                pZ2 = psZ.tile([R, H2], F32, tag="z2")
                for k in range(2):
                    nc.tensor.matmul(pZ2[:], lhsT=AnNbrT[:, k, R * b:R * b + R],
                                     rhs=X1sb[:, k, :], start=(k == 0), stop=(k == 1))
                Z2Sb = wl.tile([R, H2], BF16, tag="Z2S")
                nc.vector.tensor_copy(Z2Sb[:], pZ2[:])
                Z2s[b] = Z2Sb

            def s5(b):   # transpose to feature-major Z2T columns
                Z2Sb = Z2s.pop(b)
                pZT = psT.tile([128, 4, R], BF16, tag="zt")
                for m in range(4):
                    nc.tensor.transpose(pZT[:, m, :], Z2Sb[:, 128 * m:128 * (m + 1)],
                                        identb[0:R, 0:R])
                nc.vector.tensor_copy(Z2T_all[:, :, R * b:R * b + R], pZT[:])

            STAGES = [s0, s1, s2, s3, s4, s5]
            OFF = [0, 1, 2, 3, 4, 5]
            if stage == 3:
                STAGES, OFF = STAGES[:3], OFF[:3]
            elif stage == 4:
                STAGES, OFF = STAGES[:4], OFF[:4]
            for v in range(S + OFF[-1]):
                for f, off in zip(STAGES, OFF):
                    b = v - off
                    if 0 <= b < S:
                        f(b)
            if stage == 3:
                stage_out(Y1Bufs[(S - 1) % 2][0:1, 0:S])
            elif stage == 4:
                stage_out(X1s[S - 1][0:1, 0, 0:S])

        if stage in (3, 4):
            return
        if stage == 5:
            stage_out(Z2T_all[0:1, 0, 0:S])
            return

        # ---------------- batched W2 + weighted reduce -----------------------
        X2S_all = cp.tile([128, NCH, H2], BF16)
        with tc.tile_pool(name="psW2", bufs=3, space="PSUM") as psW2:
            for j in range(NCH):
                pW2 = psW2.tile([128, H2], F32, tag="w2")
                nc.tensor.matmul(pW2[:], lhsT=onesrow[:], rhs=b2row[:],
                                 start=True, stop=False)
                for k in range(4):
                    nc.tensor.matmul(pW2[:], lhsT=Z2T_all[:, k, 128 * j:128 * (j + 1)],
                                     rhs=W2sb[:, k, :], start=False, stop=(k == 3))
                if j % 2 == 0:
                    nc.scalar.activation(X2S_all[:, j, :], pW2[:], ACTF.Relu)
                else:
                    nc.vector.tensor_scalar_max(X2S_all[:, j, :], pW2[:], 0.0)
            if stage == 6:
                stage_out(X2S_all[0:1, 0, 0:S])
                return

            # R = WSel^T @ X2S  [S, 512]
            pR = psW2.tile([S, H2], F32, tag="r")
            for j in range(NCH):
                nc.tensor.matmul(pR[:], lhsT=WSel[:, j, :], rhs=X2S_all[:, j, :],
                                 start=(j == 0), stop=(j == NCH - 1))
            Rb = cp.tile([S, H2], BF16)
            nc.scalar.activation(Rb[:], pR[:], ACTF.Copy)

        with tc.tile_pool(name="psEnd", bufs=2, space="PSUM") as psEnd:
            # RT [128, 4, S] feature-major
            pRT = psEnd.tile([128, 4, S], BF16, tag="rt")
            for m in range(4):
                nc.tensor.transpose(pRT[:, m, :], Rb[:, 128 * m:128 * (m + 1)],
                                    identb[0:S, 0:S])
            RTb = cp.tile([128, 4, S], BF16)
            nc.vector.tensor_copy(RTb[:], pRT[:])
            if stage == 7:
                stage_out(RTb[0:1, 0, 0:S])
                return

            # ---------------- G3 = relu(R @ W3 + b3), H = G3 @ fcW1 + fcb1 ---
            G3T = cp.tile([128, 8, S], BF16)
            for mb in range(8):
                pG3 = psEnd.tile([128, S], F32, tag="g3")
                for k in range(4):
                    nc.tensor.matmul(pG3[:], lhsT=W3sb[:, k, 128 * mb:128 * (mb + 1)],
                                     rhs=RTb[:, k, :], start=(k == 0), stop=(k == 3))
                nc.scalar.activation(G3T[:, mb, :], pG3[:], ACTF.Relu,
                                     bias=b3c[:, mb:mb + 1])
            HT = cp.tile([128, 4, S], F32)
            for m in range(4):
                pH = psEnd.tile([128, S], F32, tag="h")
                for k in range(8):
                    nc.tensor.matmul(pH[:], lhsT=fcW1sb[:, k, 128 * m:128 * (m + 1)],
                                     rhs=G3T[:, k, :], start=(k == 0), stop=(k == 7))
                nc.scalar.activation(HT[:, m, :], pH[:], ACTF.Identity,
                                     bias=fcb1c[:, m:m + 1])

            # ---------------- BatchNorm stats + AllReduce --------------------
            stats = cp.tile([128, 8], F32)
            sjunk = cp.tile([128, S], F32)
            for m in range(4):
                nc.vector.tensor_reduce(stats[:, m:m + 1], HT[:, m, :], axis=AX.X,
                                        op=OP.add)
                nc.scalar.activation(sjunk[:], HT[:, m, :], ACTF.Square,
                                     accum_out=stats[:, 4 + m:5 + m])
            if stage == 8:
                stage_out(stats[0:1, 0:S])
                return
            cc_in = dp.tile([128, 8], F32)
            cc_out = dp.tile([128, 8], F32)
            nc.sync.dma_start(cc_in[:], stats[:])
            nc.gpsimd.collective_compute(
                "AllReduce", OP.add, replica_groups=[list(range(N_CORES))],
                ins=[cc_in.opt()], outs=[cc_out.opt()])
            statsG = cp.tile([128, 8], F32)
            nc.sync.dma_start(statsG[:], cc_out[:])

            mean = cp.tile([128, 4], F32)
            ex2 = cp.tile([128, 4], F32)
            var = cp.tile([128, 4], F32)
            sd = cp.tile([128, 4], F32)
            rstd = cp.tile([128, 4], F32)
            scl = cp.tile([128, 4], F32)
            sft = cp.tile([128, 4], F32)
            nc.vector.tensor_scalar_mul(mean[:], statsG[:, 0:4], 1.0 / BATCH)
            nc.vector.tensor_scalar_mul(ex2[:], statsG[:, 4:8], 1.0 / BATCH)
            nc.vector.tensor_tensor(var[:], mean[:], mean[:], op=OP.mult)
            nc.vector.tensor_tensor(var[:], ex2[:], var[:], op=OP.subtract)
            epsc = cp.tile([128, 1], F32)
            nc.gpsimd.memset(epsc[:], BN_EPS)
            nc.scalar.activation(sd[:], var[:], ACTF.Sqrt, bias=epsc[:, 0:1])
            nc.vector.reciprocal(rstd[:], sd[:])
            nc.vector.tensor_tensor(scl[:], gammac[:], rstd[:], op=OP.mult)
            nc.vector.tensor_tensor(sft[:], mean[:], scl[:], op=OP.mult)
            nc.vector.tensor_tensor(sft[:], betac[:], sft[:], op=OP.subtract)

            # normalize + leaky relu, then FC2 + sigmoid
            Hl = cp.tile([128, 4, S], BF16)
            Hn = cp.tile([128, S], F32)
            for m in range(4):
                nc.scalar.activation(Hn[:], HT[:, m, :], ACTF.Identity,
                                     scale=scl[:, m:m + 1], bias=sft[:, m:m + 1])
                nc.vector.scalar_tensor_tensor(
                    out=Hl[:, m, :], in0=Hn[:], scalar=LEAKY, in1=Hn[:],
                    op0=OP.mult, op1=OP.max)
            pO = psEnd.tile([1, S], F32, tag="o")
            for c in range(4):
                nc.tensor.matmul(pO[:], lhsT=fcW2sb[:, c:c + 1], rhs=Hl[:, c, :],
                                 start=(c == 0), stop=(c == 3))
            outT = cp.tile([1, S], F32)
            nc.scalar.activation(outT[:], pO[:], ACTF.Sigmoid, bias=fcb2t[0:1, 0:1])
            nc.sync.dma_start(out_d.rearrange("b o -> o b"), outT[:])


_NC_CACHE = {}
_LAST_RESULT = None


def _get_nc(S: int, R: int):
    key = (S, R)
    if key not in _NC_CACHE:
        _NC_CACHE[key] = build_nc(S, R)
    return _NC_CACHE[key]


def _host_structure(x_slice, Abar_pattern, S, R):
    """Build one-hot index tensors (pure structure, no float math)."""
    COLS = S * R
    N = Abar_pattern.shape[0]
    g = x_slice[:, -1, -2].astype(np.int64)
    E = np.zeros((N, COLS), np.float32)
    G1h = np.zeros((N, S), np.float32)
    Msk = np.zeros((COLS, S), np.float32)
    for b in range(S):
        nbr = np.nonzero(Abar_pattern[g[b]])[0]
        cnt = len(nbr)
        E[nbr, R * b + np.arange(cnt)] = 1.0
        G1h[g[b], b] = 1.0
        Msk[R * b:R * b + cnt, b] = 1.0
    return E, G1h, Msk


def kernel(**inputs) -> np.ndarray:
    S = BATCH // N_CORES
    full_x = np.ascontiguousarray(inputs["x"], dtype=np.float32)
    adj = np.ascontiguousarray(inputs["adj_mat"], dtype=np.float32)
    Abar_pattern = (adj + np.eye(NODE, dtype=np.float32)) > 0
    max_nbr = int(Abar_pattern[full_x[:, -1, -2].astype(np.int64)].sum(1).max())
    R = 40 if max_nbr <= 40 else (48 if max_nbr <= 48 else 64)
    assert max_nbr <= 64, f"degree {max_nbr} exceeds kernel capacity"
    nc = _get_nc(S, R)

    shared = {}
    for k in ("adj_mat", "W2", "b2", "W3", "b3", "fcW1", "fcb1",
              "gamma", "beta", "fcW2", "fcb2"):
        shared[k] = np.ascontiguousarray(inputs[k], dtype=np.float32)
    W1b1 = np.zeros((96, H1), np.float32)
    W1b1[0:FEAT] = inputs["W1"]
    W1b1[64] = np.asarray(inputs["b1"])
    shared["W1b1"] = W1b1
    in_maps = []
    for c in range(N_CORES):
        m = dict(shared)
        xs = np.ascontiguousarray(full_x[c * S:(c + 1) * S])
        m["x"] = xs
        E, G1h, Msk = _host_structure(xs, Abar_pattern, S, R)
        m["E"], m["G1h"], m["Msk"] = E, G1h, Msk
        in_maps.append(m)
    res = bass_utils.run_bass_kernel_spmd(
        nc, in_maps, core_ids=list(range(N_CORES)))
    global _LAST_RESULT
    _LAST_RESULT = res
    out = np.concatenate([res.results[c]["out"] for c in range(N_CORES)], axis=0)
    return out.astype(np.float32)


if __name__ == "__main__":
    print("building...")
    nc = _get_nc(BATCH // N_CORES, 40)
    print("built ok")


# revision 15
# speedup vs baseline: 1.6603x; 1.0602x over previous
"""GCN2 Trainium2 kernel: 3-layer GCN + FC head with BatchNorm, 8-core data-parallel.

Self-contained: hardcodes shapes from the problem spec.
  x [256, 128, 65] f32, adj_mat [256, 256] f32, W1 [63, 512], b1 [512],
  W2 [512, 512], b2 [512], W3 [512, 1024], b3 [1024], fcW1 [1024, 512],
  fcb1 [512], gamma [512], beta [512], fcW2 [512, 1], fcb2 [1] -> out [256, 1]

Sharding: batch 256 -> 32 samples per core on 8 cores; weights/adj replicated.
BatchNorm batch stats all-reduced across cores (one small [128,8] AllReduce).

Algorithm (sparse tail):
  The output gather X3[b, g_b] means layers >= 2 are only needed at the
  neighbors of g_b (max degree+1 = R slots per sample). Per sample:
    X1 = relu(An @ scatter(F) @ W1 + b1)          dense [256, 512]
    Z2 = An[nbr(g), :] @ X1                       [R, 512]
    X2 = relu(Z2 @ W2 + b2)                       [R, 512]
    r  = An[g, nbr(g)] @ X2                       [512]
  then batched W3/FC head over the 32 samples.

  All index gathers are expressed as matmuls against one-hot matrices so the
  kernel uses NO indirect DMA:
    scatter(F)       = S_b^T @ F with S_b[j, n] = (sid[j] == n)  (on-device iota)
    An[:, nbr-cols]  = An @ E,  E one-hot neighbor columns       (host 0/1)
    An[g, nbr] terms = (E^T @ An @ G1h) * Msk                    (host 0/1)
  E/G1h/Msk encode only index structure (no float math on host).
"""
import os
import sys

if "/opt/trn_rl_repo" not in sys.path:
    sys.path.insert(0, "/opt/trn_rl_repo")

import numpy as np

import concourse.bass as bass
import concourse.mybir as mybir
import concourse.tile as tile
from concourse import bacc, bass_utils
from concourse.masks import make_identity

N_CORES = 8
BATCH, NODE, SEQ, FEAT = 256, 256, 128, 63   # FEAT = feature_num - 1
H1, H2, H3, FC = 512, 512, 1024, 512
BN_EPS = 1e-5
LEAKY = 0.01

F32 = mybir.dt.float32
BF16 = mybir.dt.bfloat16
I32 = mybir.dt.int32
AX = mybir.AxisListType
OP = mybir.AluOpType
ACTF = mybir.ActivationFunctionType


def build_nc(S: int, R: int):
    """Build the SPMD kernel for S samples per core, R neighbor slots."""
    COLS = S * R
    assert COLS % 128 == 0
    nc = bacc.Bacc("TRN2", target_bir_lowering=False, debug=False,
                   num_devices=N_CORES)

    x_d = nc.dram_tensor("x", [S, SEQ, FEAT + 2], F32, kind="ExternalInput").ap()
    adj_d = nc.dram_tensor("adj_mat", [NODE, NODE], F32, kind="ExternalInput").ap()
    W1b1_d = nc.dram_tensor("W1b1", [96, H1], F32, kind="ExternalInput").ap()
    W2_d = nc.dram_tensor("W2", [H1, H2], F32, kind="ExternalInput").ap()
    b2_d = nc.dram_tensor("b2", [H2], F32, kind="ExternalInput").ap()
    W3_d = nc.dram_tensor("W3", [H2, H3], F32, kind="ExternalInput").ap()
    b3_d = nc.dram_tensor("b3", [H3], F32, kind="ExternalInput").ap()
    fcW1_d = nc.dram_tensor("fcW1", [H3, FC], F32, kind="ExternalInput").ap()
    fcb1_d = nc.dram_tensor("fcb1", [FC], F32, kind="ExternalInput").ap()
    gamma_d = nc.dram_tensor("gamma", [FC], F32, kind="ExternalInput").ap()
    beta_d = nc.dram_tensor("beta", [FC], F32, kind="ExternalInput").ap()
    fcW2_d = nc.dram_tensor("fcW2", [FC, 1], F32, kind="ExternalInput").ap()
    fcb2_d = nc.dram_tensor("fcb2", [1], F32, kind="ExternalInput").ap()
    E_d = nc.dram_tensor("E", [NODE, COLS], F32, kind="ExternalInput").ap()
    G1h_d = nc.dram_tensor("G1h", [NODE, S], F32, kind="ExternalInput").ap()
    Msk_d = nc.dram_tensor("Msk", [COLS, S], F32, kind="ExternalInput").ap()
    out_d = nc.dram_tensor("out", [S, 1], F32, kind="ExternalOutput").ap()

    with tile.TileContext(nc) as tc:
        _body(nc, tc, S, R, x_d, adj_d, W1b1_d, W2_d, b2_d, W3_d, b3_d,
              fcW1_d, fcb1_d, gamma_d, beta_d, fcW2_d, fcb2_d,
              E_d, G1h_d, Msk_d, out_d)
    nc.compile()
    return nc


def _body(nc, tc, S, R, x_d, adj_d, W1b1_d, W2_d, b2_d, W3_d, b3_d,
          fcW1_d, fcb1_d, gamma_d, beta_d, fcW2_d, fcb2_d,
          E_d, G1h_d, Msk_d, out_d):
    COLS = S * R
    NCH = COLS // 128
    stage = int(os.environ.get("BISECT_STAGE", "0"))

    def stage_out(ap2d):
        # dump a [1, S] row to the output for bisection
        nc.sync.dma_start(out_d.rearrange("b o -> o b"), ap2d)

    with tc.tile_pool(name="const", bufs=1) as cp, \
         tc.tile_pool(name="dram", bufs=1, space="DRAM") as dp:

        # ---------------- constants & weights --------------------------------
        ident = cp.tile([128, 128], F32)
        make_identity(nc, ident[:])
        identb = cp.tile([128, 128], BF16)
        nc.vector.tensor_copy(identb[:], ident[:])
        iotaI = cp.tile([128, NODE], I32)
        nc.gpsimd.iota(iotaI[:], pattern=[[1, NODE]], base=0, channel_multiplier=0)
        iotaF = cp.tile([128, NODE], F32)
        nc.vector.tensor_copy(iotaF[:], iotaI[:])
        onesrow = cp.tile([1, 128], BF16)
        nc.gpsimd.memset(onesrow[:], 1.0)

        # DMA order matters: the gpsimd queue serializes all casting DMAs, so
        # issue what the loop needs first (x, E, W1, W2) and the late-phase
        # W3/fcW1 tensors last; f32 loads go on other queues.
        Fall = cp.tile([128, S, FEAT + 2], F32)   # [seq, sample, feat]
        nc.sync.dma_start(Fall[:], x_d.rearrange("b j f -> j b f"))
        A0 = cp.tile([128, 2, NODE], F32)
        nc.scalar.dma_start(A0[:], adj_d.rearrange("(c p) n -> p c n", p=128))
        Esb = cp.tile([128, 2, COLS], BF16)
        nc.gpsimd.dma_start(Esb[:], E_d.rearrange("(k p) c -> p k c", p=128))
        G1sb = cp.tile([128, 2, S], BF16)
        nc.gpsimd.dma_start(G1sb[:], G1h_d.rearrange("(k p) b -> p k b", p=128))
        Msksb = cp.tile([128, NCH, S], BF16)
        nc.gpsimd.dma_start(Msksb[:], Msk_d.rearrange("(j p) b -> p j b", p=128))
        W1b1sb = cp.tile([96, H1], BF16)
        nc.gpsimd.dma_start(W1b1sb[:], W1b1_d[:])
        W2sb = cp.tile([128, 4, H2], BF16)
        nc.gpsimd.dma_start(W2sb[:], W2_d.rearrange("(k p) f -> p k f", p=128))
        b2row = cp.tile([1, H2], BF16)
        nc.gpsimd.dma_start(b2row[:], b2_d[None, :])
        fcW2sb = cp.tile([128, 4], BF16)
        nc.gpsimd.dma_start(fcW2sb[:], fcW2_d.rearrange("(c p) o -> p (c o)", p=128))
        W3sb = cp.tile([128, 4, H3], BF16)
        nc.gpsimd.dma_start(W3sb[:], W3_d.rearrange("(k p) h -> p k h", p=128))
        fcW1sb = cp.tile([128, 8, FC], BF16)
        nc.gpsimd.dma_start(fcW1sb[:], fcW1_d.rearrange("(k p) f -> p k f", p=128))
        b3c = cp.tile([128, 8], F32)
        nc.sync.dma_start(b3c[:], b3_d.rearrange("(m p) -> p m", p=128))
        fcb1c = cp.tile([128, 4], F32)
        nc.sync.dma_start(fcb1c[:], fcb1_d.rearrange("(m p) -> p m", p=128))
        gammac = cp.tile([128, 4], F32)
        nc.sync.dma_start(gammac[:], gamma_d.rearrange("(m p) -> p m", p=128))
        betac = cp.tile([128, 4], F32)
        nc.sync.dma_start(betac[:], beta_d.rearrange("(m p) -> p m", p=128))
        fcb2t = cp.tile([1, 1], F32)
        nc.sync.dma_start(fcb2t[:], fcb2_d[None, :])

        # Prewarm the collective channels during the prologue so the real
        # BN AllReduce at the tail doesn't pay the cold-start cost.
        warm_in = dp.tile([1, 8], F32)
        warm_out = dp.tile([1, 8], F32)
        warm_sb = cp.tile([1, 8], F32)
        nc.gpsimd.memset(warm_sb[:], 0.0)
        nc.sync.dma_start(warm_in[:], warm_sb[:])
        nc.gpsimd.collective_compute(
            "AllReduce", OP.add, replica_groups=[list(range(N_CORES))],
            ins=[warm_in.opt()], outs=[warm_out.opt()])

        # ---------------- adjacency normalization ----------------------------
        # An = diag(dis) (A + I) diag(dis),  dis = 1/sqrt(rowsum(A) + 1)
        rs = cp.tile([128, 2], F32)
        for c in range(2):
            nc.vector.tensor_reduce(rs[:, c:c + 1], A0[:, c, :], axis=AX.X, op=OP.add)
        sq = cp.tile([128, 2], F32)
        nc.scalar.activation(sq[:], rs[:], ACTF.Sqrt, bias=1.0)
        dis = cp.tile([128, 2], F32)
        nc.vector.reciprocal(dis[:], sq[:])
        dis2 = cp.tile([128, 2], F32)
        nc.vector.tensor_tensor(dis2[:], dis[:], dis[:], op=OP.mult)
        Csc = cp.tile([128, 2, NODE], F32)
        for c in range(2):
            nc.vector.tensor_scalar_mul(Csc[:, c, :], A0[:, c, :], dis[:, c:c + 1])
        Anb = cp.tile([128, 2, NODE], BF16)
        with tc.tile_pool(name="psPro", bufs=2, space="PSUM") as psP:
            for cd in range(2):
                for cs in range(2):
                    pT = psP.tile([128, 128], F32, tag="tr")
                    nc.tensor.transpose(pT[:], Csc[:, cs, 128 * cd:128 * (cd + 1)],
                                        ident[:])
                    nc.scalar.activation(Anb[:, cd, 128 * cs:128 * (cs + 1)], pT[:],
                                         ACTF.Copy, scale=dis[:, cd:cd + 1])
        diagb = cp.tile([128, 2, NODE], BF16)
        for c in range(2):
            nc.gpsimd.affine_select(
                out=diagb[:, c, :], in_=dis2[:, c:c + 1].to_broadcast([128, NODE]),
                pattern=[[-1, NODE]], compare_op=OP.is_equal, fill=0.0,
                base=128 * c, channel_multiplier=1)
            nc.vector.tensor_tensor(Anb[:, c, :], Anb[:, c, :], diagb[:, c, :],
                                    op=OP.add)
        if stage == 1:
            stage_out(Anb[0:1, 0, 0:S])
            return

        # ---------------- neighbor-column gathers as matmuls -----------------
        # AnNbrT[:, k, c] = An[128k+p, nbr(c)] = (An @ E) chunk
        AnNbrT = cp.tile([128, 2, COLS], BF16)
        with tc.tile_pool(name="psE", bufs=2, space="PSUM") as psE:
            for m in range(2):
                for cb in range(0, COLS, 512):
                    w = min(512, COLS - cb)
                    pE = psE.tile([128, 512], F32, tag="e")
                    for k in range(2):
                        nc.tensor.matmul(pE[:, 0:w],
                                         lhsT=Anb[:, k, 128 * m:128 * (m + 1)],
                                         rhs=Esb[:, k, cb:cb + w],
                                         start=(k == 0), stop=(k == 1))
                    if (m * 3 + cb // 512) % 2 == 0:
                        nc.scalar.activation(AnNbrT[:, m, cb:cb + w], pE[:, 0:w], ACTF.Copy)
                    else:
                        nc.vector.tensor_copy(AnNbrT[:, m, cb:cb + w], pE[:, 0:w])

            # AnG1h = An @ G1h  [256, S];  WSel = (E^T @ AnG1h) * Msk
            AnG1sb = cp.tile([128, 2, S], BF16)
            pG = psE.tile([128, 2, S], F32, tag="g")
            for m in range(2):
                for k in range(2):
                    nc.tensor.matmul(pG[:, m, :],
                                     lhsT=Anb[:, k, 128 * m:128 * (m + 1)],
                                     rhs=G1sb[:, k, :],
                                     start=(k == 0), stop=(k == 1))
            nc.vector.tensor_copy(AnG1sb[:], pG[:])
            WSel = cp.tile([128, NCH, S], BF16)
            for j in range(NCH):
                pW = psE.tile([128, S], F32, tag="w")
                for k in range(2):
                    nc.tensor.matmul(pW[:], lhsT=Esb[:, k, 128 * j:128 * (j + 1)],
                                     rhs=AnG1sb[:, k, :],
                                     start=(k == 0), stop=(k == 1))
                nc.vector.tensor_tensor(WSel[:, j, :], pW[:], Msksb[:, j, :],
                                        op=OP.mult)
        if stage == 2:
            stage_out(AnNbrT[0:1, 0, 0:S])
            return

        # ---------------- per-sample pipeline --------------------------------
        Z2T_all = cp.tile([128, 4, COLS], BF16)
        FbBufs = []
        Y1Bufs = []
        for i in range(2):
            fb = cp.tile([128, FEAT + 1], BF16, tag=f"Fb{i}")
            nc.gpsimd.memset(fb[:, FEAT:FEAT + 1], 0.0)
            FbBufs.append(fb)
            y1 = cp.tile([96, NODE], BF16, tag=f"Y1aug{i}")
            nc.gpsimd.memset(y1[64:96, :], 0.0)
            nc.gpsimd.memset(y1[64:65, :], 1.0)
            Y1Bufs.append(y1)
        with tc.tile_pool(name="wl", bufs=2) as wl, \
             tc.tile_pool(name="psX0", bufs=2, space="PSUM") as psX0, \
             tc.tile_pool(name="psY", bufs=2, space="PSUM") as psY, \
             tc.tile_pool(name="psX1", bufs=1, space="PSUM") as psX1, \
             tc.tile_pool(name="psZ", bufs=1, space="PSUM") as psZ, \
             tc.tile_pool(name="psT", bufs=1, space="PSUM") as psT:
            # Software-pipelined across samples: stage s of sample b is
            # emitted at virtual step v = b + OFF[s], so each PSUM->SBUF
            # evacuation has a full step of other samples' matmuls to hide
            # its latency before its consumer runs.
            Sbs, X0s, X1s, Z2s = {}, {}, {}, {}
            pX0s, pY1s, pX1s, pZ2s, pZTs = {}, {}, {}, {}, {}

            def s0(b):   # input prep (Pool + DVE)
                Fb = FbBufs[b % 2]
                nc.gpsimd.tensor_copy(Fb[:, 0:FEAT], Fall[:, b, 0:FEAT])
                Sb = wl.tile([128, NODE], BF16, tag="Sb")
                nc.vector.tensor_scalar(
                    out=Sb[:], in0=iotaF[:], scalar1=Fall[:, b, FEAT:FEAT + 1],
                    scalar2=None, op0=OP.is_equal)
                Sbs[b] = Sb

            def s1(b):   # scatter: X0 = S_b^T @ F  [256, 64] node-major
                Fb, Sb = FbBufs[b % 2], Sbs.pop(b)
                pX0 = psX0.tile([128, 2, FEAT + 1], F32, tag="x0")
                for k in range(2):
                    nc.tensor.matmul(pX0[:, k, :], lhsT=Sb[:, 128 * k:128 * (k + 1)],
                                     rhs=Fb[:], start=True, stop=True)
                X0sb = wl.tile([128, 2, FEAT + 1], BF16, tag="X0")
                nc.vector.tensor_copy(X0sb[:], pX0[:])
                X0s[b] = X0sb

            def s2(b):   # Y1T = X0^T @ An  [64, 256] feature-major
                X0sb = X0s.pop(b)
                pY1 = psY.tile([64, NODE], F32, tag="y1")
                for k in range(2):
                    nc.tensor.matmul(pY1[:], lhsT=X0sb[:, k, :],
                                     rhs=Anb[:, k, :], start=(k == 0), stop=(k == 1))
                Y1aug = Y1Bufs[b % 2]
                nc.scalar.activation(Y1aug[0:64, :], pY1[:], ACTF.Copy)

            def s3(b):   # X1 = relu(Y1 @ W1 + b1)  [256, 512] node-major
                Y1aug = Y1Bufs[b % 2]
                pX1 = psX1.tile([128, 2, H1], F32, tag="x1")
                for t in range(2):
                    nc.tensor.matmul(pX1[:, t, :],
                                     lhsT=Y1aug[:, 128 * t:128 * (t + 1)],
                                     rhs=W1b1sb[:], start=True, stop=True)
                X1sb = wl.tile([128, 2, H1], BF16, tag="X1")
                nc.scalar.activation(X1sb[:], pX1[:], ACTF.Relu)
                X1s[b] = X1sb

            def s4(b):   # Z2S = An[nbr,:] @ X1  [R, 512]
                X1sb = X1s.pop(b)
                pZ2 = psZ.tile([R, H2], F32, tag="z2")
                for k in range(2):
                    nc.tensor.matmul(pZ2[:], lhsT=AnNbrT[:, k, R * b:R * b + R],
                                     rhs=X1sb[:, k, :], start=(k == 0), stop=(k == 1))
                Z2Sb = wl.tile([R, H2], BF16, tag="Z2S")
                nc.vector.tensor_copy(Z2Sb[:], pZ2[:])
                Z2s[b] = Z2Sb

            def s5(b):   # transpose to feature-major Z2T columns
                Z2Sb = Z2s.pop(b)
                pZT = psT.tile([128, 4, R], BF16, tag="zt")
                for m in range(4):
                    nc.tensor.transpose(pZT[:, m, :], Z2Sb[:, 128 * m:128 * (m + 1)],
                                        identb[0:R, 0:R])
                nc.vector.tensor_copy(Z2T_all[:, :, R * b:R * b + R], pZT[:])

            STAGES = [s0, s1, s2, s3, s4, s5]
            OFF = [0, 1, 2, 3, 4, 5]
            if stage == 3:
                STAGES, OFF = STAGES[:3], OFF[:3]
            elif stage == 4:
                STAGES, OFF = STAGES[:4], OFF[:4]
            for v in range(S + OFF[-1]):
                for f, off in zip(STAGES, OFF):
                    b = v - off
                    if 0 <= b < S:
                        f(b)
            if stage == 3:
                stage_out(Y1Bufs[(S - 1) % 2][0:1, 0:S])
            elif stage == 4:
                stage_out(X1s[S - 1][0:1, 0, 0:S])

        if stage in (3, 4):
            return
        if stage == 5:
            stage_out(Z2T_all[0:1, 0, 0:S])
            return

        # ---------------- batched W2 + weighted reduce -----------------------
        X2S_all = cp.tile([128, NCH, H2], BF16)
        with tc.tile_pool(name="psW2", bufs=3, space="PSUM") as psW2:
            for j in range(NCH):
                pW2 = psW2.tile([128, H2], F32, tag="w2")
                nc.tensor.matmul(pW2[:], lhsT=onesrow[:], rhs=b2row[:],
                                 start=True, stop=False)
                for k in range(4):
                    nc.tensor.matmul(pW2[:], lhsT=Z2T_all[:, k, 128 * j:128 * (j + 1)],
                                     rhs=W2sb[:, k, :], start=False, stop=(k == 3))
                if j % 2 == 0:
                    nc.scalar.activation(X2S_all[:, j, :], pW2[:], ACTF.Relu)
                else:
                    nc.vector.tensor_scalar_max(X2S_all[:, j, :], pW2[:], 0.0)
            if stage == 6:
                stage_out(X2S_all[0:1, 0, 0:S])
                return

            # R = WSel^T @ X2S  [S, 512]
            pR = psW2.tile([S, H2], F32, tag="r")
            for j in range(NCH):
                nc.tensor.matmul(pR[:], lhsT=WSel[:, j, :], rhs=X2S_all[:, j, :],
                                 start=(j == 0), stop=(j == NCH - 1))
            Rb = cp.tile([S, H2], BF16)
            nc.scalar.activation(Rb[:], pR[:], ACTF.Copy)

        with tc.tile_pool(name="psEnd", bufs=2, space="PSUM") as psEnd:
            # RT [128, 4, S] feature-major
            pRT = psEnd.tile([128, 4, S], BF16, tag="rt")
            for m in range(4):
                nc.tensor.transpose(pRT[:, m, :], Rb[:, 128 * m:128 * (m + 1)],
                                    identb[0:S, 0:S])
            RTb = cp.tile([128, 4, S], BF16)
            nc.vector.tensor_copy(RTb[:], pRT[:])
            if stage == 7:
                stage_out(RTb[0:1, 0, 0:S])
                return

            # ---------------- G3 = relu(R @ W3 + b3), H = G3 @ fcW1 + fcb1 ---
            G3T = cp.tile([128, 8, S], BF16)
            for mb in range(8):
                pG3 = psEnd.tile([128, S], F32, tag="g3")
                for k in range(4):
                    nc.tensor.matmul(pG3[:], lhsT=W3sb[:, k, 128 * mb:128 * (mb + 1)],
                                     rhs=RTb[:, k, :], start=(k == 0), stop=(k == 3))
                nc.scalar.activation(G3T[:, mb, :], pG3[:], ACTF.Relu,
                                     bias=b3c[:, mb:mb + 1])
            HT = cp.tile([128, 4, S], F32)
            for m in range(4):
                pH = psEnd.tile([128, S], F32, tag="h")
                for k in range(8):
                    nc.tensor.matmul(pH[:], lhsT=fcW1sb[:, k, 128 * m:128 * (m + 1)],
                                     rhs=G3T[:, k, :], start=(k == 0), stop=(k == 7))
                nc.scalar.activation(HT[:, m, :], pH[:], ACTF.Identity,
                                     bias=fcb1c[:, m:m + 1])

            # ---------------- BatchNorm stats + AllReduce --------------------
            stats = cp.tile([128, 8], F32)
            sjunk = cp.tile([128, S], F32)
            for m in range(4):
                nc.vector.tensor_reduce(stats[:, m:m + 1], HT[:, m, :], axis=AX.X,
                                        op=OP.add)
                nc.scalar.activation(sjunk[:], HT[:, m, :], ACTF.Square,
                                     accum_out=stats[:, 4 + m:5 + m])
            if stage == 8:
                stage_out(stats[0:1, 0:S])
                return
            cc_in = dp.tile([128, 8], F32)
            cc_out = dp.tile([128, 8], F32)
            nc.sync.dma_start(cc_in[:], stats[:])
            nc.gpsimd.collective_compute(
                "AllReduce", OP.add, replica_groups=[list(range(N_CORES))],
                ins=[cc_in.opt()], outs=[cc_out.opt()])
            statsG = cp.tile([128, 8], F32)
            nc.sync.dma_start(statsG[:], cc_out[:])

            mean = cp.tile([128, 4], F32)
            ex2 = cp.tile([128, 4], F32)
            var = cp.tile([128, 4], F32)
            sd = cp.tile([128, 4], F32)
            rstd = cp.tile([128, 4], F32)
            scl = cp.tile([128, 4], F32)
            sft = cp.tile([128, 4], F32)
            nc.vector.tensor_scalar_mul(mean[:], statsG[:, 0:4], 1.0 / BATCH)
            nc.vector.tensor_scalar_mul(ex2[:], statsG[:, 4:8], 1.0 / BATCH)
            nc.vector.tensor_tensor(var[:], mean[:], mean[:], op=OP.mult)
            nc.vector.tensor_tensor(var[:], ex2[:], var[:], op=OP.subtract)
            epsc = cp.tile([128, 1], F32)
            nc.gpsimd.memset(epsc[:], BN_EPS)
            nc.scalar.activation(sd[:], var[:], ACTF.Sqrt, bias=epsc[:, 0:1])
            nc.vector.reciprocal(rstd[:], sd[:])
            nc.vector.tensor_tensor(scl[:], gammac[:], rstd[:], op=OP.mult)
            nc.vector.tensor_tensor(sft[:], mean[:], scl[:], op=OP.mult)
            nc.vector.tensor_tensor(sft[:], betac[:], sft[:], op=OP.subtract)

            # normalize + leaky relu, then FC2 + sigmoid
            Hl = cp.tile([128, 4, S], BF16)
            Hn = cp.tile([128, S], F32)
            for m in range(4):
                nc.scalar.activation(Hn[:], HT[:, m, :], ACTF.Identity,
                                     scale=scl[:, m:m + 1], bias=sft[:, m:m + 1])
                nc.vector.scalar_tensor_tensor(
                    out=Hl[:, m, :], in0=Hn[:], scalar=LEAKY, in1=Hn[:],
                    op0=OP.mult, op1=OP.max)
            pO = psEnd.tile([1, S], F32, tag="o")
            for c in range(4):
                nc.tensor.matmul(pO[:], lhsT=fcW2sb[:, c:c + 1], rhs=Hl[:, c, :],
                                 start=(c == 0), stop=(c == 3))
            outT = cp.tile([1, S], F32)
            nc.scalar.activation(outT[:], pO[:], ACTF.Sigmoid, bias=fcb2t[0:1, 0:1])
            nc.sync.dma_start(out_d.rearrange("b o -> o b"), outT[:])


_NC_CACHE = {}
_LAST_RESULT = None


def _get_nc(S: int, R: int):
    key = (S, R)
    if key not in _NC_CACHE:
        _NC_CACHE[key] = build_nc(S, R)
    return _NC_CACHE[key]


def _host_structure(x_slice, Abar_pattern, S, R):
    """Build one-hot index tensors (pure structure, no float math)."""
    COLS = S * R
    N = Abar_pattern.shape[0]
    g = x_slice[:, -1, -2].astype(np.int64)
    E = np.zeros((N, COLS), np.float32)
    G1h = np.zeros((N, S), np.float32)
    Msk = np.zeros((COLS, S), np.float32)
    for b in range(S):
        nbr = np.nonzero(Abar_pattern[g[b]])[0]
        cnt = len(nbr)
        E[nbr, R * b + np.arange(cnt)] = 1.0
        G1h[g[b], b] = 1.0
        Msk[R * b:R * b + cnt, b] = 1.0
    return E, G1h, Msk


def kernel(**inputs) -> np.ndarray:
    S = BATCH // N_CORES
    full_x = np.ascontiguousarray(inputs["x"], dtype=np.float32)
    adj = np.ascontiguousarray(inputs["adj_mat"], dtype=np.float32)
    Abar_pattern = (adj + np.eye(NODE, dtype=np.float32)) > 0
    max_nbr = int(Abar_pattern[full_x[:, -1, -2].astype(np.int64)].sum(1).max())
    R = 40 if max_nbr <= 40 else (48 if max_nbr <= 48 else 64)
    assert max_nbr <= 64, f"degree {max_nbr} exceeds kernel capacity"
    nc = _get_nc(S, R)

    shared = {}
    for k in ("adj_mat", "W2", "b2", "W3", "b3", "fcW1", "fcb1",
              "gamma", "beta", "fcW2", "fcb2"):
        shared[k] = np.ascontiguousarray(inputs[k], dtype=np.float32)
    W1b1 = np.zeros((96, H1), np.float32)
    W1b1[0:FEAT] = inputs["W1"]
    W1b1[64] = np.asarray(inputs["b1"])
    shared["W1b1"] = W1b1
    in_maps = []
    for c in range(N_CORES):
        m = dict(shared)
        xs = np.ascontiguousarray(full_x[c * S:(c + 1) * S])
        m["x"] = xs
        E, G1h, Msk = _host_structure(xs, Abar_pattern, S, R)
        m["E"], m["G1h"], m["Msk"] = E, G1h, Msk
        in_maps.append(m)
    res = bass_utils.run_bass_kernel_spmd(
        nc, in_maps, core_ids=list(range(N_CORES)))
    global _LAST_RESULT
    _LAST_RESULT = res
    out = np.concatenate([res.results[c]["out"] for c in range(N_CORES)], axis=0)
    return out.astype(np.float32)


if __name__ == "__main__":
    print("building...")
    nc = _get_nc(BATCH // N_CORES, 40)
    print("built ok")
